# revision 1
# baseline (speedup 1.0000x reference)
import sys

sys.path.insert(0, "/opt/trn_rl_repo")

import numpy as np
import ml_dtypes

import concourse.bass as bass
import concourse.bass_isa as bass_isa
from concourse import bacc
import concourse.mybir as mybir
import concourse.tile as tile
from concourse.bass_utils import run_bass_kernel_spmd

# Problem constants (hardcoded; see nn_ConvLSTMAutoencoder spec)
B_TOT, T, F = 128, 100, 64
NCORES = 8
B = B_TOT // NCORES          # 16 batch per core (pure data parallelism)
SEG = F + 2                  # 66: spatial row stored with 1 zero pad each side
C0, C1 = 16, 32              # encoder hidden dims; decoder mirrors [32, 16]

F32 = mybir.dt.float32
BF16 = mybir.dt.bfloat16

DT_ACT = F32    # arena / h / matmul inputs + weights
NP_ACT = ml_dtypes.bfloat16 if DT_ACT == BF16 else np.float32

Tanh = mybir.ActivationFunctionType.Tanh
MULT = mybir.AluOpType.mult
ADD = mybir.AluOpType.add

# Hardware rules (TRN2, probed via birverifier): every compute-engine
# operand must start at partition 0/32/64/96; spans may not cross upward
# into a lower-numbered quadrant boundary (base 0: any count, base 64:
# up to 64, bases 32/96: up to 32); both inputs of a 2-input op must
# share the same start. One-input ops (activation / tensor_scalar /
# copies) may re-base. DMA is unconstrained.
#
# Gate layout along M for each conv matmul ("spread"):
#   C=32: [i 0:32 | f 32:64 | o 64:96 | g 96:128]            M = 128
#   C=16: [i 0:16 |-| f 32:48 |-| o 64:80 |-| g 96:112]      M = 112
# i/f/o weight+bias rows are pre-halved so one tanh gives all gates:
#   th = tanh(z)         (g rows: true tanh(g); i/f/o rows: tanh(./2))
#   s  = 0.5*th+0.5 over rows 0:64+C -> sigma(i)@0, sigma(f)@32, sigma(o)@64
#   ghat re-based to 0 by a gpsimd copy. Cell state lives at rows 32:32+C
#   of its own tile; tanh(c) re-based to 64 to meet sigma(o).

MM_NB = 8        # batches per matmul instruction (8*64 = 512 <= psum bank)


def _conv_taps(nc, zt, wt, rhs_src):
    """3-tap conv as PSUM-accumulated matmuls."""
    for d in range(3):
        rhs = rhs_src(d)
        for nb in range(0, B, MM_NB):
            nc.tensor.matmul(
                zt[:, nb:nb + MM_NB, :],
                wt[:, d, :],
                rhs[:, nb:nb + MM_NB, :],
                start=(d == 0),
                stop=(d == 2),
            )


def _lstm_cell(nc, wpool, z, C, M, bvec, ctile, h_out):
    """Gate math for one ConvLSTM step under the alignment rules.

    z: PSUM [M, B, F] spread as documented above. bvec: SBUF [M,1].
    ctile: persistent [32+C, B, F], state in rows 32:32+C. h_out: arena AP.
    """
    th = wpool.tile([M, B, F], F32, tag="th", name="th")
    nc.scalar.activation(th[:], z[:], Tanh, bias=bvec[:])
    s = wpool.tile([64 + C, B, F], F32, tag="s", name="s")
    # sigma = 0.5*tanh(z/2)+0.5 for the i/f/o rows (g rows untouched)
    nc.vector.tensor_scalar(s[:], th[0:64 + C], 0.5, 0.5, MULT, ADD)
    gh = wpool.tile([C, B, F], F32, tag="gh", name="gh")
    nc.gpsimd.tensor_scalar_mul(gh[:], th[96:96 + C], 1.0)        # ghat -> base 0
    u = wpool.tile([C, B, F], F32, tag="u", name="u")
    nc.vector.tensor_tensor(u[:], s[0:C], gh[:], MULT)            # sig(i)*ghat
    v = wpool.tile([C, B, F], F32, tag="v", name="v")
    nc.vector.tensor_tensor(v[:], s[32:32 + C], ctile[32:32 + C], MULT)
    nc.vector.tensor_tensor(ctile[32:32 + C], u[:], v[:], ADD)    # c updated
    tc_ = wpool.tile([64 + C, B, F], F32, tag="tc", name="tc")
    nc.scalar.activation(tc_[64:64 + C], ctile[32:32 + C], Tanh)
    nc.vector.tensor_tensor(h_out, s[64:64 + C], tc_[64:64 + C], MULT)
    return th, s, gh


DEBUG_TAPS = False


def build_program():
    nc = bacc.Bacc(None)
    dbg = {}
    if DEBUG_TAPS:
        for nm, shp in [("dbg_x", [1, B, SEG]), ("dbg_th0", [112, B, F]),
                        ("dbg_h0", [C0, B, SEG]), ("dbg_th1", [128, B, F]),
                        ("dbg_h1", [C1, B, SEG]), ("dbg_h0b", [C0, B, SEG]),
                        ("dbg_s0", [64 + C0, B, F]), ("dbg_gh0", [C0, B, F]),
                        ("dbg_e2", [C1, B, SEG]), ("dbg_hd0", [C1, B, SEG]),
                        ("dbg_hd1", [C0 + 1, B, SEG]), ("dbg_fci", [C0 + 1, B, F]),
                        ("dbg_ofc", [1, B, F])]:
            dbg[nm] = nc.declare_dram_parameter(nm, shp, F32, isOutput=True)

    x_pad = nc.declare_dram_parameter("x_pad", [T, B, SEG], DT_ACT, isOutput=False)
    w0 = nc.declare_dram_parameter("w0", [1 + C0, 3, 112], DT_ACT, isOutput=False)
    w1 = nc.declare_dram_parameter("w1", [2 * C1, 3, 128], DT_ACT, isOutput=False)
    wd0 = nc.declare_dram_parameter("wd0", [2 * C1, 3, 128], DT_ACT, isOutput=False)
    wd1 = nc.declare_dram_parameter("wd1", [2 * C1 + C0, 3, 112], DT_ACT, isOutput=False)
    fcv = nc.declare_dram_parameter("fcv", [C0 + 1, 1], F32, isOutput=False)
    b0 = nc.declare_dram_parameter("b0", [112, 1], F32, isOutput=False)
    b1 = nc.declare_dram_parameter("b1", [128, 1], F32, isOutput=False)
    bd0 = nc.declare_dram_parameter("bd0", [128, 1], F32, isOutput=False)
    bd1 = nc.declare_dram_parameter("bd1", [112, 1], F32, isOutput=False)
    out = nc.declare_dram_parameter("out", [B, T, F], F32, isOutput=True)

    with tile.TileContext(nc) as tc:
        with (
            tc.tile_pool(name="const", bufs=1) as cpool,
            tc.tile_pool(name="state", bufs=1) as spool,
            tc.tile_pool(name="work", bufs=2) as wpool,
            tc.tile_pool(name="zp", bufs=2, space="PSUM") as zpool,
            tc.tile_pool(name="fcp", bufs=2, space="PSUM") as fcpool,
        ):
            w0t = cpool.tile([1 + C0, 3, 112], DT_ACT)
            w1t = cpool.tile([2 * C1, 3, 128], DT_ACT)
            wd0t = cpool.tile([2 * C1, 3, 128], DT_ACT)
            # dec1 reads arena_d[0:80] with zero weights for the e2 rows
            wd1t = cpool.tile([2 * C1 + C0, 3, 112], DT_ACT)
            fcvt = cpool.tile([C0 + 1, 1], F32)
            b0t = cpool.tile([112, 1], F32)
            b1t = cpool.tile([128, 1], F32)
            bd0t = cpool.tile([128, 1], F32)
            bd1t = cpool.tile([112, 1], F32)
            for dst, dsrc in [(w0t[:], w0), (w1t[:], w1), (wd0t[:], wd0),
                              (wd1t[:], wd1), (fcvt[:], fcv), (b0t[:], b0),
                              (b1t[:], b1), (bd0t[:], bd0), (bd1t[:], bd1)]:
                nc.sync.dma_start(dst, dsrc[:])

            # Encoder arena rows: 0:16 h_enc0 | 16 x_t | 17:32 zero | 32:64
            # h_enc1.  enc0 reads [0:17], enc1 reads [0:64] with zero weight
            # rows for x/junk.
            # Decoder arena rows: 0:32 enc2_t | 32:64 h_dec0 | 64:80 h_dec1 |
            # 80 ones (carries fc bias through the gpsimd fc reduce).
            arena_e = spool.tile([2 * C1, B, SEG], DT_ACT)
            arena_d = spool.tile([2 * C1 + C0 + 1, B, SEG], DT_ACT)
            seq = spool.tile([128, (T + 3) // 4, B, F], DT_ACT)
            nc.vector.memset(arena_e[:], 0.0)
            nc.vector.memset(arena_d[0:64], 0.0)
            nc.vector.memset(arena_d[64:81], 1.0)   # row 80 stays 1.0
            nc.vector.memset(arena_d[64:80], 0.0)

            # cell states live at rows 32:32+C of their own tiles
            ce0 = spool.tile([32 + C0, B, F], F32)
            ce1 = spool.tile([32 + C1, B, F], F32)
            cd0 = spool.tile([32 + C1, B, F], F32)
            cd1 = spool.tile([32 + C0, B, F], F32)
            nc.vector.memset(ce0[32:32 + C0], 0.0)
            nc.vector.memset(ce1[32:32 + C1], 0.0)

            # ---------------- encoder ----------------
            for t in range(T):
                nc.gpsimd.dma_start(arena_e[C0:C0 + 1, :, :], x_pad[t:t + 1, :, :])

                z0 = zpool.tile([112, B, F], F32, tag="z", name="z0")
                _conv_taps(nc, z0, w0t,
                           lambda d: arena_e[0:1 + C0, :, d:d + F])
                th0, s0, gh0 = _lstm_cell(nc, wpool, z0, C0, 112, b0t, ce0,
                                          arena_e[0:C0, :, 1:1 + F])

                z1 = zpool.tile([128, B, F], F32, tag="z", name="z1")
                _conv_taps(nc, z1, w1t,
                           lambda d: arena_e[0:2 * C1, :, d:d + F])
                th1, _, _ = _lstm_cell(nc, wpool, z1, C1, 128, b1t, ce1,
                                       arena_e[C1:2 * C1, :, 1:1 + F])

                if DEBUG_TAPS and t == 0:
                    nc.sync.dma_start(dbg["dbg_x"][:], arena_e[C0:C0 + 1, :, :])
                    nc.sync.dma_start(dbg["dbg_th0"][:], th0[:])
                    nc.sync.dma_start(dbg["dbg_s0"][:], s0[:])
                    nc.sync.dma_start(dbg["dbg_gh0"][:], gh0[:])
                    nc.sync.dma_start(dbg["dbg_h0"][:], arena_e[0:C0, :, :])
                    nc.sync.dma_start(dbg["dbg_th1"][:], th1[:])
                    nc.sync.dma_start(dbg["dbg_h1"][:], arena_e[C1:2 * C1, :, :])
                if DEBUG_TAPS and t == 1:
                    nc.sync.dma_start(dbg["dbg_h0b"][:], arena_e[0:C0, :, :])

                r = (t % 4) * 32
                nc.sync.dma_start(seq[r:r + 32, t // 4],
                                  arena_e[C1:2 * C1, :, 1:1 + F])

            # ---------------- decoder init ----------------
            nc.sync.dma_start(arena_d[C1:2 * C1, :, :], arena_e[C1:2 * C1, :, :])
            nc.sync.dma_start(arena_d[2 * C1:2 * C1 + C0, :, :],
                              arena_e[0:C0, :, :])
            nc.sync.dma_start(cd0[32:32 + C1], ce1[32:32 + C1])
            nc.sync.dma_start(cd1[32:32 + C0], ce0[32:32 + C0])

            # ---------------- decoder ----------------
            for t in range(T):
                r = (t % 4) * 32
                nc.gpsimd.dma_start(arena_d[0:C1, :, 1:1 + F],
                                    seq[r:r + 32, t // 4])

                zd0 = zpool.tile([128, B, F], F32, tag="z", name="zd0")
                _conv_taps(nc, zd0, wd0t,
                           lambda d: arena_d[0:2 * C1, :, d:d + F])
                _lstm_cell(nc, wpool, zd0, C1, 128, bd0t, cd0,
                           arena_d[C1:2 * C1, :, 1:1 + F])

                zd1 = zpool.tile([112, B, F], F32, tag="z", name="zd1")
                _conv_taps(nc, zd1, wd1t,
                           lambda d: arena_d[0:2 * C1 + C0, :, d:d + F])
                _lstm_cell(nc, wpool, zd1, C0, 112, bd1t, cd1,
                           arena_d[2 * C1:2 * C1 + C0, :, 1:1 + F])

                # final 1x1 conv: rebase [h_dec1; ones] to base 0 on
                # gpsimd, then a plain K=17, M=1 matmul (ones row carries
                # fc_b), ACT copy out of PSUM.
                fci = wpool.tile([C0 + 1, B, F], F32, tag="fci", name="fci")
                nc.gpsimd.tensor_scalar_mul(
                    fci[:], arena_d[2 * C1:2 * C1 + C0 + 1, :, 1:1 + F], 1.0)
                zfc = fcpool.tile([1, B, F], F32, tag="fc", name="zfc")
                for nb in range(0, B, MM_NB):
                    nc.tensor.matmul(zfc[:, nb:nb + MM_NB, :], fcvt[:],
                                     fci[:, nb:nb + MM_NB, :],
                                     start=True, stop=True)
                ofc = wpool.tile([1, B, F], F32, tag="ofc", name="ofc")
                nc.scalar.mul(ofc[:], zfc[:], 1.0)
                nc.sync.dma_start(out[:, t, :], ofc[0:1, :, :])

                if DEBUG_TAPS and t == 0:
                    nc.sync.dma_start(dbg["dbg_e2"][:], arena_d[0:C1, :, :])
                    nc.sync.dma_start(dbg["dbg_hd0"][:], arena_d[C1:2 * C1, :, :])
                    nc.sync.dma_start(dbg["dbg_hd1"][:],
                                      arena_d[2 * C1:2 * C1 + C0 + 1, :, :])
                    nc.sync.dma_start(dbg["dbg_fci"][:], fci[:])
                    nc.sync.dma_start(dbg["dbg_ofc"][:], ofc[:])

    nc.finalize()
    return nc


# M-column spread per gate, by hidden size
def _m_cols(C):
    return {"i": 0, "f": 32, "o": 64, "g": 96}, 96 + C


def _prep_weights(w, b, Cin, C, row_order):
    """[4C, Cin, 3, 3] -> lhsT [len(row_order), 3, M] with the gate spread.

    Reference gate order along output channels is i, f, o, g. i/f/o rows are
    halved (sigmoid computed as 0.5*tanh(z/2)+0.5). row_order maps lhsT
    row -> input channel (-1 = zero row).
    """
    cols, M = _m_cols(C)
    w3 = np.asarray(w, np.float32).reshape(4 * C, Cin, 3, 3)[:, :, :, 1]
    b = np.asarray(b, np.float32).reshape(4 * C)
    gate_of = {"i": 0, "f": 1, "o": 2, "g": 3}
    lhsT = np.zeros((len(row_order), 3, M), np.float32)
    bvec = np.zeros((M, 1), np.float32)
    for gname, col0 in cols.items():
        gi = gate_of[gname]
        scale = 0.5 if gname in ("i", "f", "o") else 1.0
        for j in range(C):
            oc = gi * C + j
            bvec[col0 + j, 0] = b[oc] * scale
            for r, ch in enumerate(row_order):
                if ch >= 0:
                    lhsT[r, :, col0 + j] = w3[oc, ch, :] * scale
    return np.ascontiguousarray(lhsT).astype(NP_ACT), bvec


_CACHE = {}


def kernel(x, enc_w0, enc_b0, enc_w1, enc_b1, dec_w0, dec_b0, dec_w1, dec_b1,
           fc_w, fc_b):
    if "nc" not in _CACHE:
        _CACHE["nc"] = build_program()
    nc = _CACHE["nc"]

    x = np.asarray(x, np.float32)
    # enc0 rhs rows: [h0 (channels 1..16); x (channel 0)]
    w0, b0 = _prep_weights(enc_w0, enc_b0, 1 + C0, C0,
                           row_order=list(range(1, 17)) + [0])
    # enc1 rhs rows: [h0 (ch 0..15); x row; 15 junk rows; h1 (ch 16..47)]
    w1, b1 = _prep_weights(enc_w1, enc_b1, C0 + C1, C1,
                           row_order=list(range(16)) + [-1] * 16 +
                           list(range(16, 48)))
    wd0, bd0 = _prep_weights(dec_w0, dec_b0, C1 + C1, C1,
                             row_order=list(range(64)))
    wd1, bd1 = _prep_weights(dec_w1, dec_b1, C1 + C0, C0,
                             row_order=[-1] * 32 + list(range(48)))
    fcv = np.concatenate(
        [np.asarray(fc_w, np.float32).reshape(C0),
         np.asarray(fc_b, np.float32).reshape(1)]).reshape(C0 + 1, 1)
    fcv = np.ascontiguousarray(fcv.astype(np.float32))

    in_maps = []
    for core in range(NCORES):
        xs = x[core * B:(core + 1) * B]      # [B, T, F]
        xp = np.zeros((T, B, SEG), np.float32)
        xp[:, :, 1:1 + F] = xs.transpose(1, 0, 2)
        in_maps.append({
            "x_pad": xp.astype(NP_ACT),
            "w0": w0, "w1": w1, "wd0": wd0, "wd1": wd1, "fcv": fcv,
            "b0": b0, "b1": b1, "bd0": bd0, "bd1": bd1,
        })

    _CACHE["in_maps"] = in_maps
    res = run_bass_kernel_spmd(nc, in_maps, core_ids=list(range(NCORES)))
    outs = [res.results[i]["out"] for i in range(NCORES)]
    return np.concatenate(outs, axis=0).astype(np.float32)


if __name__ == "__main__":
    rng = np.random.default_rng(0)
    inputs = {
        "x": rng.standard_normal((B_TOT, T, F), dtype=np.float32),
        "enc_w0": rng.standard_normal((4 * C0, 1 + C0, 3, 3), dtype=np.float32) * 0.05,
        "enc_b0": np.zeros(4 * C0, np.float32),
        "enc_w1": rng.standard_normal((4 * C1, C0 + C1, 3, 3), dtype=np.float32) * 0.05,
        "enc_b1": np.zeros(4 * C1, np.float32),
        "dec_w0": rng.standard_normal((4 * C1, C1 + C1, 3, 3), dtype=np.float32) * 0.05,
        "dec_b0": np.zeros(4 * C1, np.float32),
        "dec_w1": rng.standard_normal((4 * C0, C1 + C0, 3, 3), dtype=np.float32) * 0.05,
        "dec_b1": np.zeros(4 * C0, np.float32),
        "fc_w": rng.standard_normal((1, C0, 1, 1), dtype=np.float32) * 0.05,
        "fc_b": np.zeros(1, np.float32),
    }
    out = kernel(**inputs)
    print("out", out.shape, out.dtype, np.abs(out).max())



# revision 3
# speedup vs baseline: 3.5752x; 3.5752x over previous
import sys

sys.path.insert(0, "/opt/trn_rl_repo")

import numpy as np
import ml_dtypes

import concourse.bass as bass
import concourse.bass_isa as bass_isa
from concourse import bacc
import concourse.mybir as mybir
import concourse.tile as tile
from concourse.bass_utils import run_bass_kernel_spmd

# Problem constants (hardcoded; see nn_ConvLSTMAutoencoder spec)
B_TOT, T, F = 128, 100, 64
NCORES = 8
B = B_TOT // NCORES          # 16 batch per core (pure data parallelism)
SEG = F + 2                  # 66: spatial row stored with 1 zero pad each side
C0, C1 = 16, 32              # encoder hidden dims; decoder mirrors [32, 16]

F32 = mybir.dt.float32
BF16 = mybir.dt.bfloat16
NP_BF16 = ml_dtypes.bfloat16

Tanh = mybir.ActivationFunctionType.Tanh
Sigmoid = mybir.ActivationFunctionType.Sigmoid
MULT = mybir.AluOpType.mult
ADD = mybir.AluOpType.add

# Hardware rules (TRN2, probed via birverifier + perfetto): every
# compute-engine operand must start at partition 0/32/64/96; spans may not
# cross upward into a lower-numbered quadrant boundary (base 0: any count,
# base 64: up to 64, bases 32/96: up to 32). Both inputs of a 2-input op
# must share the same start. One-input ops (activation / tensor_scalar /
# copies) may re-base. DMA is unconstrained.
#
# Perf rules measured on HW: a 2-input DVE op whose sources sit at
# partition base 32 runs ~12x slower (14.5us vs 1.2us) than at base 0/64,
# and gpsimd tensor ops cost ~15us each. So: cell state c lives at base 0,
# every 2-input DVE op reads at base 0 or 64 only, and all gate rebasing
# is done by scalar-engine activations (which are fast at any base).
#
# Gate layout along M for each conv matmul ("spread"):
#   C=32: [i 0:32 | f 32:64 | o 64:96 | g 96:128]            M = 128
#   C=16: [i 0:16 |-| f 32:48 |-| o 64:80 |-| g 96:112]      M = 112
# Per cell:
#   sg[0:64+C]  = Sigmoid(z[0:64+C] + b)   (covers i@0 and o@64; f rows junk)
#   sf[0:C]     = Sigmoid(z[32:32+C] + b_f)  scalar re-base 32 -> 0
#   gh[0:C]     = Tanh(z[96:96+C] + b_g)     scalar re-base 96 -> 0
#   u = sg[0:C] * gh ; v = sf * c ; c = u + v          (DVE, all base 0)
#   tc[64:64+C] = Tanh(c)                    scalar re-base 0 -> 64
#   h = sg[64:64+C] * tc[64:64+C]            (DVE, base 64)
# Matmul operands (weights, arena, x, seq) are bf16: 1 cycle/row vs 4 for
# fp32, half the SBUF/DMA footprint. PSUM accumulation stays fp32.

MM_NB = 8        # batches per matmul instruction (8*64 = 512 <= psum bank)


def _conv_taps(nc, zt, wt, rhs_src):
    """3-tap conv as PSUM-accumulated matmuls (bf16 operands)."""
    for d in range(3):
        rhs = rhs_src(d)
        for nb in range(0, B, MM_NB):
            nc.tensor.matmul(
                zt[:, nb:nb + MM_NB, :],
                wt[:, d, :],
                rhs[:, nb:nb + MM_NB, :],
                start=(d == 0),
                stop=(d == 2),
            )


def _lstm_cell(nc, wpool, z, C, M, bvec, ctile, h_out):
    """Gate math for one ConvLSTM step under the alignment + perf rules.

    z: PSUM [M, B, F] spread as documented above. bvec: SBUF [M,1].
    ctile: persistent [C, B, F] cell state at base 0. h_out: arena AP.
    """
    gh = wpool.tile([C, B, F], F32, tag="gh", name="gh")
    nc.scalar.activation(gh[:], z[96:96 + C], Tanh, bias=bvec[96:96 + C])
    sg = wpool.tile([64 + C, B, F], F32, tag="sg", name="sg")
    nc.scalar.activation(sg[:], z[0:64 + C], Sigmoid, bias=bvec[0:64 + C])
    sf = wpool.tile([C, B, F], F32, tag="sf", name="sf")
    nc.scalar.activation(sf[:], z[32:32 + C], Sigmoid, bias=bvec[32:32 + C])
    u = wpool.tile([C, B, F], F32, tag="u", name="u")
    nc.vector.tensor_tensor(u[:], sg[0:C], gh[:], MULT)      # sig(i)*tanh(g)
    v = wpool.tile([C, B, F], F32, tag="v", name="v")
    nc.vector.tensor_tensor(v[:], sf[:], ctile[:], MULT)     # sig(f)*c
    nc.vector.tensor_tensor(ctile[:], u[:], v[:], ADD)       # c updated
    tc_ = wpool.tile([64 + C, B, F], F32, tag="tc", name="tc")
    nc.scalar.activation(tc_[64:64 + C], ctile[:], Tanh)
    nc.vector.tensor_tensor(h_out, sg[64:64 + C], tc_[64:64 + C], MULT)


def build_program():
    nc = bacc.Bacc(None)

    x_pad = nc.declare_dram_parameter("x_pad", [T, B, SEG], BF16, isOutput=False)
    w0 = nc.declare_dram_parameter("w0", [1 + C0, 3, 112], BF16, isOutput=False)
    w1 = nc.declare_dram_parameter("w1", [2 * C1, 3, 128], BF16, isOutput=False)
    wd0 = nc.declare_dram_parameter("wd0", [2 * C1, 3, 128], BF16, isOutput=False)
    wd1 = nc.declare_dram_parameter("wd1", [2 * C1 + C0, 3, 112], BF16, isOutput=False)
    fcv = nc.declare_dram_parameter("fcv", [C0 + 1, 1], BF16, isOutput=False)
    b0 = nc.declare_dram_parameter("b0", [112, 1], F32, isOutput=False)
    b1 = nc.declare_dram_parameter("b1", [128, 1], F32, isOutput=False)
    bd0 = nc.declare_dram_parameter("bd0", [128, 1], F32, isOutput=False)
    bd1 = nc.declare_dram_parameter("bd1", [112, 1], F32, isOutput=False)
    out = nc.declare_dram_parameter("out", [B, T, F], F32, isOutput=True)

    with tile.TileContext(nc) as tc:
        with (
            tc.tile_pool(name="const", bufs=1) as cpool,
            tc.tile_pool(name="state", bufs=1) as spool,
            tc.tile_pool(name="work", bufs=2) as wpool,
            tc.tile_pool(name="zp", bufs=2, space="PSUM") as zpool,
            tc.tile_pool(name="fcp", bufs=2, space="PSUM") as fcpool,
        ):
            w0t = cpool.tile([1 + C0, 3, 112], BF16)
            w1t = cpool.tile([2 * C1, 3, 128], BF16)
            wd0t = cpool.tile([2 * C1, 3, 128], BF16)
            # dec1 reads arena_d[0:80] with zero weights for the e2 rows
            wd1t = cpool.tile([2 * C1 + C0, 3, 112], BF16)
            # fc weights staged at partitions 64:81 so the fc matmul can read
            # h_dec1 (+ones row) straight from the arena at base 64.
            fcvt = cpool.tile([64 + C0 + 1, 1], BF16)
            b0t = cpool.tile([112, 1], F32)
            b1t = cpool.tile([128, 1], F32)
            bd0t = cpool.tile([128, 1], F32)
            bd1t = cpool.tile([112, 1], F32)
            for dst, dsrc in [(w0t[:], w0), (w1t[:], w1), (wd0t[:], wd0),
                              (wd1t[:], wd1), (fcvt[64:64 + C0 + 1], fcv),
                              (b0t[:], b0),
                              (b1t[:], b1), (bd0t[:], bd0), (bd1t[:], bd1)]:
                nc.sync.dma_start(dst, dsrc[:])

            # Encoder arena rows: 0:16 h_enc0 | 16 x_t | 17:32 zero | 32:64
            # h_enc1.  enc0 reads [0:17], enc1 reads [0:64] with zero weight
            # rows for x/junk.
            # Decoder arena rows: 0:32 enc2_t | 32:64 h_dec0 | 64:80 h_dec1 |
            # 80 ones (carries fc bias through the base-64 fc matmul).
            arena_e = spool.tile([2 * C1, B, SEG], BF16)
            arena_d = spool.tile([2 * C1 + C0 + 1, B, SEG], BF16)
            seq = spool.tile([128, (T + 3) // 4, B, F], BF16)
            nc.vector.memset(arena_e[:], 0.0)
            nc.vector.memset(arena_d[0:64], 0.0)
            nc.vector.memset(arena_d[64:81], 1.0)   # row 80 stays 1.0
            nc.vector.memset(arena_d[64:80], 0.0)

            # cell states at partition base 0 of their own tiles
            ce0 = spool.tile([C0, B, F], F32)
            ce1 = spool.tile([C1, B, F], F32)
            cd0 = spool.tile([C1, B, F], F32)
            cd1 = spool.tile([C0, B, F], F32)
            nc.vector.memset(ce0[:], 0.0)
            nc.vector.memset(ce1[:], 0.0)

            # ---------------- encoder ----------------
            for t in range(T):
                nc.sync.dma_start(arena_e[C0:C0 + 1, :, :], x_pad[t:t + 1, :, :])

                z0 = zpool.tile([112, B, F], F32, tag="z", name="z0")
                _conv_taps(nc, z0, w0t,
                           lambda d: arena_e[0:1 + C0, :, d:d + F])
                _lstm_cell(nc, wpool, z0, C0, 112, b0t, ce0,
                           arena_e[0:C0, :, 1:1 + F])

                z1 = zpool.tile([128, B, F], F32, tag="z", name="z1")
                _conv_taps(nc, z1, w1t,
                           lambda d: arena_e[0:2 * C1, :, d:d + F])
                _lstm_cell(nc, wpool, z1, C1, 128, b1t, ce1,
                           arena_e[C1:2 * C1, :, 1:1 + F])

                r = (t % 4) * 32
                nc.sync.dma_start(seq[r:r + 32, t // 4],
                                  arena_e[C1:2 * C1, :, 1:1 + F])

            # ---------------- decoder init ----------------
            nc.sync.dma_start(arena_d[C1:2 * C1, :, :], arena_e[C1:2 * C1, :, :])
            nc.sync.dma_start(arena_d[2 * C1:2 * C1 + C0, :, :],
                              arena_e[0:C0, :, :])
            nc.sync.dma_start(cd0[:], ce1[:])
            nc.sync.dma_start(cd1[:], ce0[:])

            # ---------------- decoder ----------------
            for t in range(T):
                r = (t % 4) * 32
                nc.sync.dma_start(arena_d[0:C1, :, 1:1 + F],
                                  seq[r:r + 32, t // 4])

                zd0 = zpool.tile([128, B, F], F32, tag="z", name="zd0")
                _conv_taps(nc, zd0, wd0t,
                           lambda d: arena_d[0:2 * C1, :, d:d + F])
                _lstm_cell(nc, wpool, zd0, C1, 128, bd0t, cd0,
                           arena_d[C1:2 * C1, :, 1:1 + F])

                zd1 = zpool.tile([112, B, F], F32, tag="z", name="zd1")
                _conv_taps(nc, zd1, wd1t,
                           lambda d: arena_d[0:2 * C1 + C0, :, d:d + F])
                _lstm_cell(nc, wpool, zd1, C0, 112, bd1t, cd1,
                           arena_d[2 * C1:2 * C1 + C0, :, 1:1 + F])

                # final 1x1 conv: K=17, M=1 matmul read directly at partition
                # base 64 (ones row carries fc_b), then ACT copy out of PSUM.
                zfc = fcpool.tile([1, B, F], F32, tag="fc", name="zfc")
                for nb in range(0, B, MM_NB):
                    nc.tensor.matmul(
                        zfc[:, nb:nb + MM_NB, :],
                        fcvt[64:64 + C0 + 1],
                        arena_d[64:64 + C0 + 1, nb:nb + MM_NB, 1:1 + F],
                        start=True, stop=True)
                ofc = wpool.tile([1, B, F], F32, tag="ofc", name="ofc")
                nc.scalar.mul(ofc[:], zfc[:], 1.0)
                nc.sync.dma_start(out[:, t, :], ofc[0:1, :, :])

    nc.finalize()
    return nc


# M-column spread per gate, by hidden size
def _m_cols(C):
    return {"i": 0, "f": 32, "o": 64, "g": 96}, 96 + C


def _prep_weights(w, b, Cin, C, row_order):
    """[4C, Cin, 3, 3] -> lhsT [len(row_order), 3, M] with the gate spread.

    Reference gate order along output channels is i, f, o, g. row_order maps
    lhsT row -> input channel (-1 = zero row).
    """
    cols, M = _m_cols(C)
    w3 = np.asarray(w, np.float32).reshape(4 * C, Cin, 3, 3)[:, :, :, 1]
    b = np.asarray(b, np.float32).reshape(4 * C)
    gate_of = {"i": 0, "f": 1, "o": 2, "g": 3}
    lhsT = np.zeros((len(row_order), 3, M), np.float32)
    bvec = np.zeros((M, 1), np.float32)
    for gname, col0 in cols.items():
        gi = gate_of[gname]
        for j in range(C):
            oc = gi * C + j
            bvec[col0 + j, 0] = b[oc]
            for r, ch in enumerate(row_order):
                if ch >= 0:
                    lhsT[r, :, col0 + j] = w3[oc, ch, :]
    return np.ascontiguousarray(lhsT).astype(NP_BF16), bvec


_CACHE = {}


def kernel(x, enc_w0, enc_b0, enc_w1, enc_b1, dec_w0, dec_b0, dec_w1, dec_b1,
           fc_w, fc_b):
    if "nc" not in _CACHE:
        _CACHE["nc"] = build_program()
    nc = _CACHE["nc"]

    x = np.asarray(x, np.float32)
    # enc0 rhs rows: [h0 (channels 1..16); x (channel 0)]
    w0, b0 = _prep_weights(enc_w0, enc_b0, 1 + C0, C0,
                           row_order=list(range(1, 17)) + [0])
    # enc1 rhs rows: [h0 (ch 0..15); x row; 15 junk rows; h1 (ch 16..47)]
    w1, b1 = _prep_weights(enc_w1, enc_b1, C0 + C1, C1,
                           row_order=list(range(16)) + [-1] * 16 +
                           list(range(16, 48)))
    wd0, bd0 = _prep_weights(dec_w0, dec_b0, C1 + C1, C1,
                             row_order=list(range(64)))
    wd1, bd1 = _prep_weights(dec_w1, dec_b1, C1 + C0, C0,
                             row_order=[-1] * 32 + list(range(48)))
    fcv = np.concatenate(
        [np.asarray(fc_w, np.float32).reshape(C0),
         np.asarray(fc_b, np.float32).reshape(1)]).reshape(C0 + 1, 1)
    fcv = np.ascontiguousarray(fcv.astype(NP_BF16))

    in_maps = []
    for core in range(NCORES):
        xs = x[core * B:(core + 1) * B]      # [B, T, F]
        xp = np.zeros((T, B, SEG), np.float32)
        xp[:, :, 1:1 + F] = xs.transpose(1, 0, 2)
        in_maps.append({
            "x_pad": xp.astype(NP_BF16),
            "w0": w0, "w1": w1, "wd0": wd0, "wd1": wd1, "fcv": fcv,
            "b0": b0, "b1": b1, "bd0": bd0, "bd1": bd1,
        })

    _CACHE["in_maps"] = in_maps
    res = run_bass_kernel_spmd(nc, in_maps, core_ids=list(range(NCORES)))
    outs = [res.results[i]["out"] for i in range(NCORES)]
    return np.concatenate(outs, axis=0).astype(np.float32)


if __name__ == "__main__":
    rng = np.random.default_rng(0)
    inputs = {
        "x": rng.standard_normal((B_TOT, T, F), dtype=np.float32),
        "enc_w0": rng.standard_normal((4 * C0, 1 + C0, 3, 3), dtype=np.float32) * 0.05,
        "enc_b0": np.zeros(4 * C0, np.float32),
        "enc_w1": rng.standard_normal((4 * C1, C0 + C1, 3, 3), dtype=np.float32) * 0.05,
        "enc_b1": np.zeros(4 * C1, np.float32),
        "dec_w0": rng.standard_normal((4 * C1, C1 + C1, 3, 3), dtype=np.float32) * 0.05,
        "dec_b0": np.zeros(4 * C1, np.float32),
        "dec_w1": rng.standard_normal((4 * C0, C1 + C0, 3, 3), dtype=np.float32) * 0.05,
        "dec_b1": np.zeros(4 * C0, np.float32),
        "fc_w": rng.standard_normal((1, C0, 1, 1), dtype=np.float32) * 0.05,
        "fc_b": np.zeros(1, np.float32),
    }
    out = kernel(**inputs)
    print("out", out.shape, out.dtype, np.abs(out).max())


# revision 5
# speedup vs baseline: 3.9621x; 1.1082x over previous
import sys

sys.path.insert(0, "/opt/trn_rl_repo")

import numpy as np
import ml_dtypes

import concourse.bass as bass
import concourse.bass_isa as bass_isa
from concourse import bacc
import concourse.mybir as mybir
import concourse.tile as tile
from concourse.bass_utils import run_bass_kernel_spmd

# Problem constants (hardcoded; see nn_ConvLSTMAutoencoder spec)
B_TOT, T, F = 128, 100, 64
NCORES = 8
B = B_TOT // NCORES          # 16 batch per core (pure data parallelism)
SEG = F + 2                  # 66: spatial row stored with 1 zero pad each side
C0, C1 = 16, 32              # encoder hidden dims; decoder mirrors [32, 16]

F32 = mybir.dt.float32
BF16 = mybir.dt.bfloat16
NP_BF16 = ml_dtypes.bfloat16

Tanh = mybir.ActivationFunctionType.Tanh
Sigmoid = mybir.ActivationFunctionType.Sigmoid
MULT = mybir.AluOpType.mult
ADD = mybir.AluOpType.add

# Hardware rules (TRN2, probed via birverifier + perfetto): every
# compute-engine operand must start at partition 0/32/64/96; spans may not
# cross upward into a lower-numbered quadrant boundary (base 0: any count,
# base 64: up to 64, bases 32/96: up to 32). Both inputs of a 2-input op
# must share the same start. One-input ops (activation / tensor_scalar /
# copies) may re-base. DMA is unconstrained.
#
# Perf rules measured on HW: a 2-input DVE op whose sources sit at
# partition base 32 runs ~12x slower (14.5us vs 1.2us) than at base 0/64,
# and gpsimd tensor ops cost ~15us each. So: cell state c lives at base 0,
# every 2-input DVE op reads at base 0 or 64 only, and all gate rebasing
# is done by scalar-engine activations (which are fast at any base).
#
# Gate layout along M for each conv matmul ("spread"):
#   C=32: [i 0:32 | f 32:64 | o 64:96 | g 96:128]            M = 128
#   C=16: [i 0:16 |-| f 32:48 |-| o 64:80 |-| g 96:112]      M = 112
# Per cell:
#   sg[0:64+C]  = Sigmoid(z[0:64+C] + b)   (covers i@0 and o@64; f rows junk)
#   sf[0:C]     = Sigmoid(z[32:32+C] + b_f)  scalar re-base 32 -> 0
#   gh[0:C]     = Tanh(z[96:96+C] + b_g)     scalar re-base 96 -> 0
#   u = sg[0:C] * gh ; v = sf * c ; c = u + v          (DVE, all base 0)
#   tc[64:64+C] = Tanh(c)                    scalar re-base 0 -> 64
#   h = sg[64:64+C] * tc[64:64+C]            (DVE, base 64)
# All gate/cell tensors are bf16 so every DVE op hits the 2x_1p fast mode.
# Matmul operands (weights, arena, x, seq) are bf16: 1 cycle/row vs 4 for
# fp32, half the SBUF/DMA footprint. PSUM accumulation stays fp32.

MM_NB = 8        # batches per matmul instruction (8*64 = 512 <= psum bank)


def _conv_taps(nc, zt, wt, rhs_src):
    """3-tap conv as PSUM-accumulated matmuls (bf16 operands)."""
    for d in range(3):
        rhs = rhs_src(d)
        for nb in range(0, B, MM_NB):
            nc.tensor.matmul(
                zt[:, nb:nb + MM_NB, :],
                wt[:, d, :],
                rhs[:, nb:nb + MM_NB, :],
                start=(d == 0),
                stop=(d == 2),
            )


def _lstm_cell(nc, wpool, z, C, M, bvec, ctile, h_out):
    """Gate math for one ConvLSTM step under the alignment + perf rules.

    z: PSUM [M, B, F] spread as documented above. bvec: SBUF [M,1].
    ctile: persistent [C, B, F] cell state at base 0. h_out: arena AP.
    """
    gh = wpool.tile([C, B, F], BF16, tag="gh", name="gh")
    nc.scalar.activation(gh[:], z[96:96 + C], Tanh, bias=bvec[96:96 + C])
    sg = wpool.tile([64 + C, B, F], BF16, tag="sg", name="sg")
    nc.scalar.activation(sg[:], z[0:64 + C], Sigmoid, bias=bvec[0:64 + C])
    sf = wpool.tile([C, B, F], BF16, tag="sf", name="sf")
    nc.scalar.activation(sf[:], z[32:32 + C], Sigmoid, bias=bvec[32:32 + C])
    u = wpool.tile([C, B, F], BF16, tag="u", name="u")
    nc.vector.tensor_tensor(u[:], sg[0:C], gh[:], MULT)      # sig(i)*tanh(g)
    v = wpool.tile([C, B, F], BF16, tag="v", name="v")
    nc.vector.tensor_tensor(v[:], sf[:], ctile[:], MULT)     # sig(f)*c
    nc.vector.tensor_tensor(ctile[:], u[:], v[:], ADD)       # c updated
    tc_ = wpool.tile([64 + C, B, F], BF16, tag="tc", name="tc")
    nc.scalar.activation(tc_[64:64 + C], ctile[:], Tanh)
    nc.vector.tensor_tensor(h_out, sg[64:64 + C], tc_[64:64 + C], MULT)


def build_program():
    nc = bacc.Bacc(None)

    x_pad = nc.declare_dram_parameter("x_pad", [T, B, SEG], BF16, isOutput=False)
    w0 = nc.declare_dram_parameter("w0", [1 + C0, 3, 112], BF16, isOutput=False)
    w1 = nc.declare_dram_parameter("w1", [2 * C1, 3, 128], BF16, isOutput=False)
    wd0 = nc.declare_dram_parameter("wd0", [2 * C1, 3, 128], BF16, isOutput=False)
    wd1 = nc.declare_dram_parameter("wd1", [2 * C1 + C0, 3, 112], BF16, isOutput=False)
    fcv = nc.declare_dram_parameter("fcv", [C0 + 1, 1], BF16, isOutput=False)
    b0 = nc.declare_dram_parameter("b0", [112, 1], F32, isOutput=False)
    b1 = nc.declare_dram_parameter("b1", [128, 1], F32, isOutput=False)
    bd0 = nc.declare_dram_parameter("bd0", [128, 1], F32, isOutput=False)
    bd1 = nc.declare_dram_parameter("bd1", [112, 1], F32, isOutput=False)
    out = nc.declare_dram_parameter("out", [B, T, F], F32, isOutput=True)

    with tile.TileContext(nc) as tc:
        with (
            tc.tile_pool(name="const", bufs=1) as cpool,
            tc.tile_pool(name="state", bufs=1) as spool,
            tc.tile_pool(name="work", bufs=2) as wpool,
            tc.tile_pool(name="zp", bufs=2, space="PSUM") as zpool,
            tc.tile_pool(name="fcp", bufs=2, space="PSUM") as fcpool,
        ):
            w0t = cpool.tile([1 + C0, 3, 112], BF16)
            w1t = cpool.tile([2 * C1, 3, 128], BF16)
            wd0t = cpool.tile([2 * C1, 3, 128], BF16)
            # dec1 reads arena_d[0:80] with zero weights for the e2 rows
            wd1t = cpool.tile([2 * C1 + C0, 3, 112], BF16)
            # fc weights staged at partitions 64:81 so the fc matmul can read
            # h_dec1 (+ones row) straight from the arena at base 64.
            fcvt = cpool.tile([64 + C0 + 1, 1], BF16)
            b0t = cpool.tile([112, 1], F32)
            b1t = cpool.tile([128, 1], F32)
            bd0t = cpool.tile([128, 1], F32)
            bd1t = cpool.tile([112, 1], F32)
            for dst, dsrc in [(w0t[:], w0), (w1t[:], w1), (wd0t[:], wd0),
                              (wd1t[:], wd1), (fcvt[64:64 + C0 + 1], fcv),
                              (b0t[:], b0),
                              (b1t[:], b1), (bd0t[:], bd0), (bd1t[:], bd1)]:
                nc.sync.dma_start(dst, dsrc[:])

            # Encoder arena rows: 0:16 h_enc0 | 16 x_t | 17:32 zero | 32:64
            # h_enc1.  enc0 reads [0:17], enc1 reads [0:64] with zero weight
            # rows for x/junk.
            # Decoder arena rows: 0:32 enc2_t | 32:64 h_dec0 | 64:80 h_dec1 |
            # 80 ones (carries fc bias through the base-64 fc matmul).
            arena_e = spool.tile([2 * C1, B, SEG], BF16)
            arena_d = spool.tile([2 * C1 + C0 + 1, B, SEG], BF16)
            seq = spool.tile([128, (T + 3) // 4, B, F], BF16)
            nc.vector.memset(arena_e[:], 0.0)
            nc.vector.memset(arena_d[0:64], 0.0)
            nc.vector.memset(arena_d[64:81], 1.0)   # row 80 stays 1.0
            nc.vector.memset(arena_d[64:80], 0.0)

            # cell states at partition base 0 of their own tiles
            ce0 = spool.tile([C0, B, F], BF16)
            ce1 = spool.tile([C1, B, F], BF16)
            cd0 = spool.tile([C1, B, F], BF16)
            cd1 = spool.tile([C0, B, F], BF16)
            nc.vector.memset(ce0[:], 0.0)
            nc.vector.memset(ce1[:], 0.0)

            # ---------------- encoder ----------------
            for t in range(T):
                nc.sync.dma_start(arena_e[C0:C0 + 1, :, :], x_pad[t:t + 1, :, :])

                z0 = zpool.tile([112, B, F], F32, tag="z", name="z0")
                _conv_taps(nc, z0, w0t,
                           lambda d: arena_e[0:1 + C0, :, d:d + F])
                _lstm_cell(nc, wpool, z0, C0, 112, b0t, ce0,
                           arena_e[0:C0, :, 1:1 + F])

                z1 = zpool.tile([128, B, F], F32, tag="z", name="z1")
                _conv_taps(nc, z1, w1t,
                           lambda d: arena_e[0:2 * C1, :, d:d + F])
                _lstm_cell(nc, wpool, z1, C1, 128, b1t, ce1,
                           arena_e[C1:2 * C1, :, 1:1 + F])

                r = (t % 4) * 32
                nc.sync.dma_start(seq[r:r + 32, t // 4],
                                  arena_e[C1:2 * C1, :, 1:1 + F])

            # ---------------- decoder init ----------------
            nc.sync.dma_start(arena_d[C1:2 * C1, :, :], arena_e[C1:2 * C1, :, :])
            nc.sync.dma_start(arena_d[2 * C1:2 * C1 + C0, :, :],
                              arena_e[0:C0, :, :])
            nc.sync.dma_start(cd0[:], ce1[:])
            nc.sync.dma_start(cd1[:], ce0[:])

            # ---------------- decoder ----------------
            for t in range(T):
                r = (t % 4) * 32
                nc.sync.dma_start(arena_d[0:C1, :, 1:1 + F],
                                  seq[r:r + 32, t // 4])

                zd0 = zpool.tile([128, B, F], F32, tag="z", name="zd0")
                _conv_taps(nc, zd0, wd0t,
                           lambda d: arena_d[0:2 * C1, :, d:d + F])
                _lstm_cell(nc, wpool, zd0, C1, 128, bd0t, cd0,
                           arena_d[C1:2 * C1, :, 1:1 + F])

                zd1 = zpool.tile([112, B, F], F32, tag="z", name="zd1")
                _conv_taps(nc, zd1, wd1t,
                           lambda d: arena_d[0:2 * C1 + C0, :, d:d + F])
                _lstm_cell(nc, wpool, zd1, C0, 112, bd1t, cd1,
                           arena_d[2 * C1:2 * C1 + C0, :, 1:1 + F])

                # final 1x1 conv: K=17, M=1 matmul read directly at partition
                # base 64 (ones row carries fc_b), then ACT copy out of PSUM.
                zfc = fcpool.tile([1, B, F], F32, tag="fc", name="zfc")
                for nb in range(0, B, MM_NB):
                    nc.tensor.matmul(
                        zfc[:, nb:nb + MM_NB, :],
                        fcvt[64:64 + C0 + 1],
                        arena_d[64:64 + C0 + 1, nb:nb + MM_NB, 1:1 + F],
                        start=True, stop=True)
                ofc = wpool.tile([1, B, F], F32, tag="ofc", name="ofc")
                nc.scalar.mul(ofc[:], zfc[:], 1.0)
                nc.sync.dma_start(out[:, t, :], ofc[0:1, :, :])

    nc.finalize()
    return nc


# M-column spread per gate, by hidden size
def _m_cols(C):
    return {"i": 0, "f": 32, "o": 64, "g": 96}, 96 + C


def _prep_weights(w, b, Cin, C, row_order):
    """[4C, Cin, 3, 3] -> lhsT [len(row_order), 3, M] with the gate spread.

    Reference gate order along output channels is i, f, o, g. row_order maps
    lhsT row -> input channel (-1 = zero row).
    """
    cols, M = _m_cols(C)
    w3 = np.asarray(w, np.float32).reshape(4 * C, Cin, 3, 3)[:, :, :, 1]
    b = np.asarray(b, np.float32).reshape(4 * C)
    gate_of = {"i": 0, "f": 1, "o": 2, "g": 3}
    lhsT = np.zeros((len(row_order), 3, M), np.float32)
    bvec = np.zeros((M, 1), np.float32)
    for gname, col0 in cols.items():
        gi = gate_of[gname]
        for j in range(C):
            oc = gi * C + j
            bvec[col0 + j, 0] = b[oc]
            for r, ch in enumerate(row_order):
                if ch >= 0:
                    lhsT[r, :, col0 + j] = w3[oc, ch, :]
    return np.ascontiguousarray(lhsT).astype(NP_BF16), bvec


_CACHE = {}


def kernel(x, enc_w0, enc_b0, enc_w1, enc_b1, dec_w0, dec_b0, dec_w1, dec_b1,
           fc_w, fc_b):
    if "nc" not in _CACHE:
        _CACHE["nc"] = build_program()
    nc = _CACHE["nc"]

    x = np.asarray(x, np.float32)
    # enc0 rhs rows: [h0 (channels 1..16); x (channel 0)]
    w0, b0 = _prep_weights(enc_w0, enc_b0, 1 + C0, C0,
                           row_order=list(range(1, 17)) + [0])
    # enc1 rhs rows: [h0 (ch 0..15); x row; 15 junk rows; h1 (ch 16..47)]
    w1, b1 = _prep_weights(enc_w1, enc_b1, C0 + C1, C1,
                           row_order=list(range(16)) + [-1] * 16 +
                           list(range(16, 48)))
    wd0, bd0 = _prep_weights(dec_w0, dec_b0, C1 + C1, C1,
                             row_order=list(range(64)))
    wd1, bd1 = _prep_weights(dec_w1, dec_b1, C1 + C0, C0,
                             row_order=[-1] * 32 + list(range(48)))
    fcv = np.concatenate(
        [np.asarray(fc_w, np.float32).reshape(C0),
         np.asarray(fc_b, np.float32).reshape(1)]).reshape(C0 + 1, 1)
    fcv = np.ascontiguousarray(fcv.astype(NP_BF16))

    in_maps = []
    for core in range(NCORES):
        xs = x[core * B:(core + 1) * B]      # [B, T, F]
        xp = np.zeros((T, B, SEG), np.float32)
        xp[:, :, 1:1 + F] = xs.transpose(1, 0, 2)
        in_maps.append({
            "x_pad": xp.astype(NP_BF16),
            "w0": w0, "w1": w1, "wd0": wd0, "wd1": wd1, "fcv": fcv,
            "b0": b0, "b1": b1, "bd0": bd0, "bd1": bd1,
        })

    _CACHE["in_maps"] = in_maps
    res = run_bass_kernel_spmd(nc, in_maps, core_ids=list(range(NCORES)))
    outs = [res.results[i]["out"] for i in range(NCORES)]
    return np.concatenate(outs, axis=0).astype(np.float32)


if __name__ == "__main__":
    rng = np.random.default_rng(0)
    inputs = {
        "x": rng.standard_normal((B_TOT, T, F), dtype=np.float32),
        "enc_w0": rng.standard_normal((4 * C0, 1 + C0, 3, 3), dtype=np.float32) * 0.05,
        "enc_b0": np.zeros(4 * C0, np.float32),
        "enc_w1": rng.standard_normal((4 * C1, C0 + C1, 3, 3), dtype=np.float32) * 0.05,
        "enc_b1": np.zeros(4 * C1, np.float32),
        "dec_w0": rng.standard_normal((4 * C1, C1 + C1, 3, 3), dtype=np.float32) * 0.05,
        "dec_b0": np.zeros(4 * C1, np.float32),
        "dec_w1": rng.standard_normal((4 * C0, C1 + C0, 3, 3), dtype=np.float32) * 0.05,
        "dec_b1": np.zeros(4 * C0, np.float32),
        "fc_w": rng.standard_normal((1, C0, 1, 1), dtype=np.float32) * 0.05,
        "fc_b": np.zeros(1, np.float32),
    }
    out = kernel(**inputs)
    print("out", out.shape, out.dtype, np.abs(out).max())


# revision 6
# speedup vs baseline: 4.0490x; 1.0219x over previous
import sys

sys.path.insert(0, "/opt/trn_rl_repo")

import numpy as np
import ml_dtypes

import concourse.bass as bass
import concourse.bass_isa as bass_isa
from concourse import bacc
import concourse.mybir as mybir
import concourse.tile as tile
from concourse.bass_utils import run_bass_kernel_spmd

# Problem constants (hardcoded; see nn_ConvLSTMAutoencoder spec)
B_TOT, T, F = 128, 100, 64
NCORES = 8
B = B_TOT // NCORES          # 16 batch per core (pure data parallelism)
SEG = F + 2                  # 66: spatial row stored with 1 zero pad each side
C0, C1 = 16, 32              # encoder hidden dims; decoder mirrors [32, 16]

F32 = mybir.dt.float32
BF16 = mybir.dt.bfloat16
NP_BF16 = ml_dtypes.bfloat16

Tanh = mybir.ActivationFunctionType.Tanh
Sigmoid = mybir.ActivationFunctionType.Sigmoid
MULT = mybir.AluOpType.mult
ADD = mybir.AluOpType.add

# Hardware rules (TRN2, probed via birverifier + perfetto): every
# compute-engine operand must start at partition 0/32/64/96; spans may not
# cross upward into a lower-numbered quadrant boundary (base 0: any count,
# base 64: up to 64, bases 32/96: up to 32). Both inputs of a 2-input op
# must share the same start. One-input ops (activation / tensor_scalar /
# copies) may re-base. DMA is unconstrained.
#
# Perf rules measured on HW: a 2-input DVE op whose sources sit at
# partition base 32 runs ~12x slower (14.5us vs 1.2us) than at base 0/64,
# and gpsimd tensor ops cost ~15us each. So: cell state c lives at base 0,
# every 2-input DVE op reads at base 0 or 64 only, and all gate rebasing
# is done by scalar-engine activations (which are fast at any base).
#
# Gate layout along M for each conv matmul ("spread"):
#   C=32: [i 0:32 | f 32:64 | o 64:96 | g 96:128]            M = 128
#   C=16: [i 0:16 |-| f 32:48 |-| o 64:80 |-| g 96:112]      M = 112
# Per cell:
#   sg[0:64+C]  = Sigmoid(z[0:64+C] + b)   (covers i@0 and o@64; f rows junk)
#   sf[0:C]     = Sigmoid(z[32:32+C] + b_f)  scalar re-base 32 -> 0
#   gh[0:C]     = Tanh(z[96:96+C] + b_g)     scalar re-base 96 -> 0
#   u = sg[0:C] * gh ; v = sf * c ; c = u + v          (DVE, all base 0)
#   tc[64:64+C] = Tanh(c)                    scalar re-base 0 -> 64
#   h = sg[64:64+C] * tc[64:64+C]            (DVE, base 64)
# All gate/cell tensors are bf16 so every DVE op hits the 2x_1p fast mode.
# Matmul operands (weights, arena, x, seq) are bf16: 1 cycle/row vs 4 for
# fp32, half the SBUF/DMA footprint. PSUM accumulation stays fp32.

MM_NB = 8        # batches per matmul instruction (8*64 = 512 <= psum bank)


def _conv_taps(nc, zt, wt, rhs_src):
    """3-tap conv as PSUM-accumulated matmuls (bf16 operands)."""
    for d in range(3):
        rhs = rhs_src(d)
        for nb in range(0, B, MM_NB):
            nc.tensor.matmul(
                zt[:, nb:nb + MM_NB, :],
                wt[:, d, :],
                rhs[:, nb:nb + MM_NB, :],
                start=(d == 0),
                stop=(d == 2),
            )


def _lstm_cell(nc, wpool, z, C, M, bvec, ctile, h_out):
    """Gate math for one ConvLSTM step under the alignment + perf rules.

    z: PSUM [M, B, F] spread as documented above. bvec: SBUF [M,1].
    ctile: persistent [C, B, F] cell state at base 0. h_out: arena AP.
    """
    gh = wpool.tile([C, B, F], BF16, tag="gh", name="gh")
    nc.scalar.activation(gh[:], z[96:96 + C], Tanh, bias=bvec[96:96 + C])
    sg = wpool.tile([64 + C, B, F], BF16, tag="sg", name="sg")
    nc.scalar.activation(sg[:], z[0:64 + C], Sigmoid, bias=bvec[0:64 + C])
    # sf: 1-input DVE copy re-basing sigma(f) from rows 32:32+C to base 0
    sf = wpool.tile([C, B, F], BF16, tag="sf", name="sf")
    nc.vector.tensor_scalar_mul(sf[:], sg[32:32 + C], 1.0)
    u = wpool.tile([C, B, F], BF16, tag="u", name="u")
    nc.vector.tensor_tensor(u[:], sg[0:C], gh[:], MULT)      # sig(i)*tanh(g)
    v = wpool.tile([C, B, F], BF16, tag="v", name="v")
    nc.vector.tensor_tensor(v[:], sf[:], ctile[:], MULT)     # sig(f)*c
    nc.vector.tensor_tensor(ctile[:], u[:], v[:], ADD)       # c updated
    tc_ = wpool.tile([64 + C, B, F], BF16, tag="tc", name="tc")
    nc.scalar.activation(tc_[64:64 + C], ctile[:], Tanh)
    nc.vector.tensor_tensor(h_out, sg[64:64 + C], tc_[64:64 + C], MULT)


def build_program():
    nc = bacc.Bacc(None)

    x_pad = nc.declare_dram_parameter("x_pad", [T, B, SEG], BF16, isOutput=False)
    w0 = nc.declare_dram_parameter("w0", [1 + C0, 3, 112], BF16, isOutput=False)
    w1 = nc.declare_dram_parameter("w1", [2 * C1, 3, 128], BF16, isOutput=False)
    wd0 = nc.declare_dram_parameter("wd0", [2 * C1, 3, 128], BF16, isOutput=False)
    wd1 = nc.declare_dram_parameter("wd1", [2 * C1 + C0, 3, 112], BF16, isOutput=False)
    fcv = nc.declare_dram_parameter("fcv", [C0 + 1, 1], BF16, isOutput=False)
    b0 = nc.declare_dram_parameter("b0", [112, 1], F32, isOutput=False)
    b1 = nc.declare_dram_parameter("b1", [128, 1], F32, isOutput=False)
    bd0 = nc.declare_dram_parameter("bd0", [128, 1], F32, isOutput=False)
    bd1 = nc.declare_dram_parameter("bd1", [112, 1], F32, isOutput=False)
    out = nc.declare_dram_parameter("out", [B, T, F], F32, isOutput=True)

    with tile.TileContext(nc) as tc:
        with (
            tc.tile_pool(name="const", bufs=1) as cpool,
            tc.tile_pool(name="state", bufs=1) as spool,
            tc.tile_pool(name="work", bufs=2) as wpool,
            tc.tile_pool(name="zp", bufs=2, space="PSUM") as zpool,
            tc.tile_pool(name="fcp", bufs=2, space="PSUM") as fcpool,
        ):
            w0t = cpool.tile([1 + C0, 3, 112], BF16)
            w1t = cpool.tile([2 * C1, 3, 128], BF16)
            wd0t = cpool.tile([2 * C1, 3, 128], BF16)
            # dec1 reads arena_d[0:80] with zero weights for the e2 rows
            wd1t = cpool.tile([2 * C1 + C0, 3, 112], BF16)
            # fc weights staged at partitions 64:81 so the fc matmul can read
            # h_dec1 (+ones row) straight from the arena at base 64.
            fcvt = cpool.tile([64 + C0 + 1, 1], BF16)
            b0t = cpool.tile([112, 1], F32)
            b1t = cpool.tile([128, 1], F32)
            bd0t = cpool.tile([128, 1], F32)
            bd1t = cpool.tile([112, 1], F32)
            for dst, dsrc in [(w0t[:], w0), (w1t[:], w1), (wd0t[:], wd0),
                              (wd1t[:], wd1), (fcvt[64:64 + C0 + 1], fcv),
                              (b0t[:], b0),
                              (b1t[:], b1), (bd0t[:], bd0), (bd1t[:], bd1)]:
                nc.sync.dma_start(dst, dsrc[:])

            # Encoder arena rows: 0:16 h_enc0 | 16 x_t | 17:32 zero | 32:64
            # h_enc1.  enc0 reads [0:17], enc1 reads [0:64] with zero weight
            # rows for x/junk.
            # Decoder arena rows: 0:32 enc2_t | 32:64 h_dec0 | 64:80 h_dec1 |
            # 80 ones (carries fc bias through the base-64 fc matmul).
            arena_e = spool.tile([2 * C1, B, SEG], BF16)
            arena_d = spool.tile([2 * C1 + C0 + 1, B, SEG], BF16)
            seq = spool.tile([128, (T + 3) // 4, B, F], BF16)
            nc.vector.memset(arena_e[:], 0.0)
            nc.vector.memset(arena_d[0:64], 0.0)
            nc.vector.memset(arena_d[64:81], 1.0)   # row 80 stays 1.0
            nc.vector.memset(arena_d[64:80], 0.0)

            # cell states at partition base 0 of their own tiles
            ce0 = spool.tile([C0, B, F], BF16)
            ce1 = spool.tile([C1, B, F], BF16)
            cd0 = spool.tile([C1, B, F], BF16)
            cd1 = spool.tile([C0, B, F], BF16)
            nc.vector.memset(ce0[:], 0.0)
            nc.vector.memset(ce1[:], 0.0)

            # ---------------- encoder ----------------
            for t in range(T):
                nc.sync.dma_start(arena_e[C0:C0 + 1, :, :], x_pad[t:t + 1, :, :])

                z0 = zpool.tile([112, B, F], F32, tag="z", name="z0")
                _conv_taps(nc, z0, w0t,
                           lambda d: arena_e[0:1 + C0, :, d:d + F])
                _lstm_cell(nc, wpool, z0, C0, 112, b0t, ce0,
                           arena_e[0:C0, :, 1:1 + F])

                z1 = zpool.tile([128, B, F], F32, tag="z", name="z1")
                _conv_taps(nc, z1, w1t,
                           lambda d: arena_e[0:2 * C1, :, d:d + F])
                _lstm_cell(nc, wpool, z1, C1, 128, b1t, ce1,
                           arena_e[C1:2 * C1, :, 1:1 + F])

                r = (t % 4) * 32
                nc.sync.dma_start(seq[r:r + 32, t // 4],
                                  arena_e[C1:2 * C1, :, 1:1 + F])

            # ---------------- decoder init ----------------
            nc.sync.dma_start(arena_d[C1:2 * C1, :, :], arena_e[C1:2 * C1, :, :])
            nc.sync.dma_start(arena_d[2 * C1:2 * C1 + C0, :, :],
                              arena_e[0:C0, :, :])
            nc.sync.dma_start(cd0[:], ce1[:])
            nc.sync.dma_start(cd1[:], ce0[:])

            # ---------------- decoder ----------------
            for t in range(T):
                r = (t % 4) * 32
                nc.sync.dma_start(arena_d[0:C1, :, 1:1 + F],
                                  seq[r:r + 32, t // 4])

                zd0 = zpool.tile([128, B, F], F32, tag="z", name="zd0")
                _conv_taps(nc, zd0, wd0t,
                           lambda d: arena_d[0:2 * C1, :, d:d + F])
                _lstm_cell(nc, wpool, zd0, C1, 128, bd0t, cd0,
                           arena_d[C1:2 * C1, :, 1:1 + F])

                zd1 = zpool.tile([112, B, F], F32, tag="z", name="zd1")
                _conv_taps(nc, zd1, wd1t,
                           lambda d: arena_d[0:2 * C1 + C0, :, d:d + F])
                _lstm_cell(nc, wpool, zd1, C0, 112, bd1t, cd1,
                           arena_d[2 * C1:2 * C1 + C0, :, 1:1 + F])

                # final 1x1 conv: K=17, M=1 matmul read directly at partition
                # base 64 (ones row carries fc_b), then ACT copy out of PSUM.
                zfc = fcpool.tile([1, B, F], F32, tag="fc", name="zfc")
                for nb in range(0, B, MM_NB):
                    nc.tensor.matmul(
                        zfc[:, nb:nb + MM_NB, :],
                        fcvt[64:64 + C0 + 1],
                        arena_d[64:64 + C0 + 1, nb:nb + MM_NB, 1:1 + F],
                        start=True, stop=True)
                ofc = wpool.tile([1, B, F], F32, tag="ofc", name="ofc")
                nc.vector.tensor_scalar_mul(ofc[:], zfc[:], 1.0)
                nc.sync.dma_start(out[:, t, :], ofc[0:1, :, :])

    nc.finalize()
    return nc


# M-column spread per gate, by hidden size
def _m_cols(C):
    return {"i": 0, "f": 32, "o": 64, "g": 96}, 96 + C


def _prep_weights(w, b, Cin, C, row_order):
    """[4C, Cin, 3, 3] -> lhsT [len(row_order), 3, M] with the gate spread.

    Reference gate order along output channels is i, f, o, g. row_order maps
    lhsT row -> input channel (-1 = zero row).
    """
    cols, M = _m_cols(C)
    w3 = np.asarray(w, np.float32).reshape(4 * C, Cin, 3, 3)[:, :, :, 1]
    b = np.asarray(b, np.float32).reshape(4 * C)
    gate_of = {"i": 0, "f": 1, "o": 2, "g": 3}
    lhsT = np.zeros((len(row_order), 3, M), np.float32)
    bvec = np.zeros((M, 1), np.float32)
    for gname, col0 in cols.items():
        gi = gate_of[gname]
        for j in range(C):
            oc = gi * C + j
            bvec[col0 + j, 0] = b[oc]
            for r, ch in enumerate(row_order):
                if ch >= 0:
                    lhsT[r, :, col0 + j] = w3[oc, ch, :]
    return np.ascontiguousarray(lhsT).astype(NP_BF16), bvec


_CACHE = {}


def kernel(x, enc_w0, enc_b0, enc_w1, enc_b1, dec_w0, dec_b0, dec_w1, dec_b1,
           fc_w, fc_b):
    if "nc" not in _CACHE:
        _CACHE["nc"] = build_program()
    nc = _CACHE["nc"]

    x = np.asarray(x, np.float32)
    # enc0 rhs rows: [h0 (channels 1..16); x (channel 0)]
    w0, b0 = _prep_weights(enc_w0, enc_b0, 1 + C0, C0,
                           row_order=list(range(1, 17)) + [0])
    # enc1 rhs rows: [h0 (ch 0..15); x row; 15 junk rows; h1 (ch 16..47)]
    w1, b1 = _prep_weights(enc_w1, enc_b1, C0 + C1, C1,
                           row_order=list(range(16)) + [-1] * 16 +
                           list(range(16, 48)))
    wd0, bd0 = _prep_weights(dec_w0, dec_b0, C1 + C1, C1,
                             row_order=list(range(64)))
    wd1, bd1 = _prep_weights(dec_w1, dec_b1, C1 + C0, C0,
                             row_order=[-1] * 32 + list(range(48)))
    fcv = np.concatenate(
        [np.asarray(fc_w, np.float32).reshape(C0),
         np.asarray(fc_b, np.float32).reshape(1)]).reshape(C0 + 1, 1)
    fcv = np.ascontiguousarray(fcv.astype(NP_BF16))

    in_maps = []
    for core in range(NCORES):
        xs = x[core * B:(core + 1) * B]      # [B, T, F]
        xp = np.zeros((T, B, SEG), np.float32)
        xp[:, :, 1:1 + F] = xs.transpose(1, 0, 2)
        in_maps.append({
            "x_pad": xp.astype(NP_BF16),
            "w0": w0, "w1": w1, "wd0": wd0, "wd1": wd1, "fcv": fcv,
            "b0": b0, "b1": b1, "bd0": bd0, "bd1": bd1,
        })

    _CACHE["in_maps"] = in_maps
    res = run_bass_kernel_spmd(nc, in_maps, core_ids=list(range(NCORES)))
    outs = [res.results[i]["out"] for i in range(NCORES)]
    return np.concatenate(outs, axis=0).astype(np.float32)


if __name__ == "__main__":
    rng = np.random.default_rng(0)
    inputs = {
        "x": rng.standard_normal((B_TOT, T, F), dtype=np.float32),
        "enc_w0": rng.standard_normal((4 * C0, 1 + C0, 3, 3), dtype=np.float32) * 0.05,
        "enc_b0": np.zeros(4 * C0, np.float32),
        "enc_w1": rng.standard_normal((4 * C1, C0 + C1, 3, 3), dtype=np.float32) * 0.05,
        "enc_b1": np.zeros(4 * C1, np.float32),
        "dec_w0": rng.standard_normal((4 * C1, C1 + C1, 3, 3), dtype=np.float32) * 0.05,
        "dec_b0": np.zeros(4 * C1, np.float32),
        "dec_w1": rng.standard_normal((4 * C0, C1 + C0, 3, 3), dtype=np.float32) * 0.05,
        "dec_b1": np.zeros(4 * C0, np.float32),
        "fc_w": rng.standard_normal((1, C0, 1, 1), dtype=np.float32) * 0.05,
        "fc_b": np.zeros(1, np.float32),
    }
    out = kernel(**inputs)
    print("out", out.shape, out.dtype, np.abs(out).max())


# revision 7
# speedup vs baseline: 4.0531x; 1.0010x over previous
import sys

sys.path.insert(0, "/opt/trn_rl_repo")

import numpy as np
import ml_dtypes

import concourse.bass as bass
import concourse.bass_isa as bass_isa
from concourse import bacc
import concourse.mybir as mybir
import concourse.tile as tile
from concourse.bass_utils import run_bass_kernel_spmd

# Problem constants (hardcoded; see nn_ConvLSTMAutoencoder spec)
B_TOT, T, F = 128, 100, 64
NCORES = 8
B = B_TOT // NCORES          # 16 batch per core (pure data parallelism)
SEG = F + 2                  # 66: spatial row stored with 1 zero pad each side
C0, C1 = 16, 32              # encoder hidden dims; decoder mirrors [32, 16]

F32 = mybir.dt.float32
BF16 = mybir.dt.bfloat16
NP_BF16 = ml_dtypes.bfloat16

Tanh = mybir.ActivationFunctionType.Tanh
Sigmoid = mybir.ActivationFunctionType.Sigmoid
MULT = mybir.AluOpType.mult
ADD = mybir.AluOpType.add

# Hardware rules (TRN2, probed via birverifier + perfetto): every
# compute-engine operand must start at partition 0/32/64/96; spans may not
# cross upward into a lower-numbered quadrant boundary (base 0: any count,
# base 64: up to 64, bases 32/96: up to 32). Both inputs of a 2-input op
# must share the same start. One-input ops (activation / tensor_scalar /
# copies) may re-base. DMA is unconstrained.
#
# Perf rules measured on HW: a 2-input DVE op whose sources sit at
# partition base 32 runs ~12x slower (14.5us vs 1.2us) than at base 0/64,
# and gpsimd tensor ops cost ~15us each. So: cell state c lives at base 0,
# every 2-input DVE op reads at base 0 or 64 only, and all gate rebasing
# is done by scalar-engine activations (which are fast at any base).
#
# Gate layout along M for each conv matmul ("spread"):
#   C=32: [i 0:32 | f 32:64 | o 64:96 | g 96:128]            M = 128
#   C=16: [i 0:16 |-| f 32:48 |-| o 64:80 |-| g 96:112]      M = 112
# Per cell:
#   sg[0:64+C]  = Sigmoid(z[0:64+C] + b)   (covers i@0 and o@64; f rows junk)
#   sf[0:C]     = Sigmoid(z[32:32+C] + b_f)  scalar re-base 32 -> 0
#   gh[0:C]     = Tanh(z[96:96+C] + b_g)     scalar re-base 96 -> 0
#   u = sg[0:C] * gh ; v = sf * c ; c = u + v          (DVE, all base 0)
#   tc[64:64+C] = Tanh(c)                    scalar re-base 0 -> 64
#   h = sg[64:64+C] * tc[64:64+C]            (DVE, base 64)
# All gate/cell tensors are bf16 so every DVE op hits the 2x_1p fast mode.
# Matmul operands (weights, arena, x, seq) are bf16: 1 cycle/row vs 4 for
# fp32, half the SBUF/DMA footprint. PSUM accumulation stays fp32.

MM_NB = 8        # batches per matmul instruction (8*64 = 512 <= psum bank)


def _conv_taps(nc, zt, wt, rhs_src):
    """3-tap conv as PSUM-accumulated matmuls (bf16 operands)."""
    for d in range(3):
        rhs = rhs_src(d)
        for nb in range(0, B, MM_NB):
            nc.tensor.matmul(
                zt[:, nb:nb + MM_NB, :],
                wt[:, d, :],
                rhs[:, nb:nb + MM_NB, :],
                start=(d == 0),
                stop=(d == 2),
            )


def _lstm_cell(nc, wpool, z, C, M, bvec, ctile, h_out):
    """Gate math for one ConvLSTM step under the alignment + perf rules.

    z: PSUM [M, B, F] spread as documented above. bvec: SBUF [M,1].
    ctile: persistent [C, B, F] cell state at base 0. h_out: arena AP.
    """
    gh = wpool.tile([C, B, F], BF16, tag="gh", name="gh")
    nc.scalar.activation(gh[:], z[96:96 + C], Tanh, bias=bvec[96:96 + C])
    sg = wpool.tile([64 + C, B, F], BF16, tag="sg", name="sg")
    nc.scalar.activation(sg[:], z[0:64 + C], Sigmoid, bias=bvec[0:64 + C])
    # sf: 1-input DVE copy re-basing sigma(f) from rows 32:32+C to base 0
    sf = wpool.tile([C, B, F], BF16, tag="sf", name="sf")
    nc.vector.tensor_scalar_mul(sf[:], sg[32:32 + C], 1.0)
    u = wpool.tile([C, B, F], BF16, tag="u", name="u")
    nc.vector.tensor_tensor(u[:], sg[0:C], gh[:], MULT)      # sig(i)*tanh(g)
    v = wpool.tile([C, B, F], BF16, tag="v", name="v")
    nc.vector.tensor_tensor(v[:], sf[:], ctile[:], MULT)     # sig(f)*c
    nc.vector.tensor_tensor(ctile[:], u[:], v[:], ADD)       # c updated
    tc_ = wpool.tile([64 + C, B, F], BF16, tag="tc", name="tc")
    nc.scalar.activation(tc_[64:64 + C], ctile[:], Tanh)
    nc.vector.tensor_tensor(h_out, sg[64:64 + C], tc_[64:64 + C], MULT)


def build_program():
    nc = bacc.Bacc(None)

    x_pad = nc.declare_dram_parameter("x_pad", [T, B, SEG], BF16, isOutput=False)
    w0 = nc.declare_dram_parameter("w0", [1 + C0, 3, 112], BF16, isOutput=False)
    w1 = nc.declare_dram_parameter("w1", [2 * C1, 3, 128], BF16, isOutput=False)
    wd0 = nc.declare_dram_parameter("wd0", [2 * C1, 3, 128], BF16, isOutput=False)
    wd1 = nc.declare_dram_parameter("wd1", [2 * C1 + C0, 3, 112], BF16, isOutput=False)
    fcv = nc.declare_dram_parameter("fcv", [C0 + 1, 1], BF16, isOutput=False)
    b0 = nc.declare_dram_parameter("b0", [112, 1], F32, isOutput=False)
    b1 = nc.declare_dram_parameter("b1", [128, 1], F32, isOutput=False)
    bd0 = nc.declare_dram_parameter("bd0", [128, 1], F32, isOutput=False)
    bd1 = nc.declare_dram_parameter("bd1", [112, 1], F32, isOutput=False)
    out = nc.declare_dram_parameter("out", [B, T, F], F32, isOutput=True)

    with tile.TileContext(nc) as tc:
        with (
            tc.tile_pool(name="const", bufs=1) as cpool,
            tc.tile_pool(name="state", bufs=1) as spool,
            tc.tile_pool(name="work", bufs=2) as wpool,
            tc.tile_pool(name="zp", bufs=1, space="PSUM") as zpool,
            tc.tile_pool(name="fcp", bufs=2, space="PSUM") as fcpool,
        ):
            w0t = cpool.tile([1 + C0, 3, 112], BF16)
            w1t = cpool.tile([2 * C1, 3, 128], BF16)
            wd0t = cpool.tile([2 * C1, 3, 128], BF16)
            # dec1 reads arena_d[0:80] with zero weights for the e2 rows
            wd1t = cpool.tile([2 * C1 + C0, 3, 112], BF16)
            # fc weights staged at partitions 64:81 so the fc matmul can read
            # h_dec1 (+ones row) straight from the arena at base 64.
            fcvt = cpool.tile([64 + C0 + 1, 1], BF16)
            b0t = cpool.tile([112, 1], F32)
            b1t = cpool.tile([128, 1], F32)
            bd0t = cpool.tile([128, 1], F32)
            bd1t = cpool.tile([112, 1], F32)
            for dst, dsrc in [(w0t[:], w0), (w1t[:], w1), (wd0t[:], wd0),
                              (wd1t[:], wd1), (fcvt[64:64 + C0 + 1], fcv),
                              (b0t[:], b0),
                              (b1t[:], b1), (bd0t[:], bd0), (bd1t[:], bd1)]:
                nc.sync.dma_start(dst, dsrc[:])

            # Encoder arena rows: 0:16 h_enc0 | 16 x_t | 17:32 zero | 32:64
            # h_enc1.  enc0 reads [0:17], enc1 reads [0:64] with zero weight
            # rows for x/junk.
            # Decoder arena rows: 0:32 enc2_t | 32:64 h_dec0 | 64:80 h_dec1 |
            # 80 ones (carries fc bias through the base-64 fc matmul).
            arena_e = spool.tile([2 * C1, B, SEG], BF16)
            arena_d = spool.tile([2 * C1 + C0 + 1, B, SEG], BF16)
            seq = spool.tile([128, (T + 3) // 4, B, F], BF16)
            nc.vector.memset(arena_e[:], 0.0)
            nc.vector.memset(arena_d[0:64], 0.0)
            nc.vector.memset(arena_d[64:81], 1.0)   # row 80 stays 1.0
            nc.vector.memset(arena_d[64:80], 0.0)

            # cell states at partition base 0 of their own tiles
            ce0 = spool.tile([C0, B, F], BF16)
            ce1 = spool.tile([C1, B, F], BF16)
            cd0 = spool.tile([C1, B, F], BF16)
            cd1 = spool.tile([C0, B, F], BF16)
            nc.vector.memset(ce0[:], 0.0)
            nc.vector.memset(ce1[:], 0.0)

            # ---------------- encoder ----------------
            for t in range(T):
                nc.sync.dma_start(arena_e[C0:C0 + 1, :, :], x_pad[t:t + 1, :, :])

                z0 = zpool.tile([112, B, F], F32, tag="z112", name="z0")
                _conv_taps(nc, z0, w0t,
                           lambda d: arena_e[0:1 + C0, :, d:d + F])
                _lstm_cell(nc, wpool, z0, C0, 112, b0t, ce0,
                           arena_e[0:C0, :, 1:1 + F])

                z1 = zpool.tile([128, B, F], F32, tag="z128", name="z1")
                _conv_taps(nc, z1, w1t,
                           lambda d: arena_e[0:2 * C1, :, d:d + F])
                _lstm_cell(nc, wpool, z1, C1, 128, b1t, ce1,
                           arena_e[C1:2 * C1, :, 1:1 + F])

                r = (t % 4) * 32
                nc.sync.dma_start(seq[r:r + 32, t // 4],
                                  arena_e[C1:2 * C1, :, 1:1 + F])

            # ---------------- decoder init ----------------
            nc.sync.dma_start(arena_d[C1:2 * C1, :, :], arena_e[C1:2 * C1, :, :])
            nc.sync.dma_start(arena_d[2 * C1:2 * C1 + C0, :, :],
                              arena_e[0:C0, :, :])
            nc.sync.dma_start(cd0[:], ce1[:])
            nc.sync.dma_start(cd1[:], ce0[:])

            # ---------------- decoder ----------------
            for t in range(T):
                r = (t % 4) * 32
                nc.sync.dma_start(arena_d[0:C1, :, 1:1 + F],
                                  seq[r:r + 32, t // 4])

                zd0 = zpool.tile([128, B, F], F32, tag="z128", name="zd0")
                _conv_taps(nc, zd0, wd0t,
                           lambda d: arena_d[0:2 * C1, :, d:d + F])
                _lstm_cell(nc, wpool, zd0, C1, 128, bd0t, cd0,
                           arena_d[C1:2 * C1, :, 1:1 + F])

                zd1 = zpool.tile([112, B, F], F32, tag="z112", name="zd1")
                _conv_taps(nc, zd1, wd1t,
                           lambda d: arena_d[0:2 * C1 + C0, :, d:d + F])
                _lstm_cell(nc, wpool, zd1, C0, 112, bd1t, cd1,
                           arena_d[2 * C1:2 * C1 + C0, :, 1:1 + F])

                # final 1x1 conv: K=17, M=1 matmul read directly at partition
                # base 64 (ones row carries fc_b), then ACT copy out of PSUM.
                zfc = fcpool.tile([1, B, F], F32, tag="fc", name="zfc")
                for nb in range(0, B, MM_NB):
                    nc.tensor.matmul(
                        zfc[:, nb:nb + MM_NB, :],
                        fcvt[64:64 + C0 + 1],
                        arena_d[64:64 + C0 + 1, nb:nb + MM_NB, 1:1 + F],
                        start=True, stop=True)
                ofc = wpool.tile([1, B, F], F32, tag="ofc", name="ofc")
                nc.vector.tensor_scalar_mul(ofc[:], zfc[:], 1.0)
                nc.sync.dma_start(out[:, t, :], ofc[0:1, :, :])

    nc.finalize()
    return nc


# M-column spread per gate, by hidden size
def _m_cols(C):
    return {"i": 0, "f": 32, "o": 64, "g": 96}, 96 + C


def _prep_weights(w, b, Cin, C, row_order):
    """[4C, Cin, 3, 3] -> lhsT [len(row_order), 3, M] with the gate spread.

    Reference gate order along output channels is i, f, o, g. row_order maps
    lhsT row -> input channel (-1 = zero row).
    """
    cols, M = _m_cols(C)
    w3 = np.asarray(w, np.float32).reshape(4 * C, Cin, 3, 3)[:, :, :, 1]
    b = np.asarray(b, np.float32).reshape(4 * C)
    gate_of = {"i": 0, "f": 1, "o": 2, "g": 3}
    lhsT = np.zeros((len(row_order), 3, M), np.float32)
    bvec = np.zeros((M, 1), np.float32)
    for gname, col0 in cols.items():
        gi = gate_of[gname]
        for j in range(C):
            oc = gi * C + j
            bvec[col0 + j, 0] = b[oc]
            for r, ch in enumerate(row_order):
                if ch >= 0:
                    lhsT[r, :, col0 + j] = w3[oc, ch, :]
    return np.ascontiguousarray(lhsT).astype(NP_BF16), bvec


_CACHE = {}


def kernel(x, enc_w0, enc_b0, enc_w1, enc_b1, dec_w0, dec_b0, dec_w1, dec_b1,
           fc_w, fc_b):
    if "nc" not in _CACHE:
        _CACHE["nc"] = build_program()
    nc = _CACHE["nc"]

    x = np.asarray(x, np.float32)
    # enc0 rhs rows: [h0 (channels 1..16); x (channel 0)]
    w0, b0 = _prep_weights(enc_w0, enc_b0, 1 + C0, C0,
                           row_order=list(range(1, 17)) + [0])
    # enc1 rhs rows: [h0 (ch 0..15); x row; 15 junk rows; h1 (ch 16..47)]
    w1, b1 = _prep_weights(enc_w1, enc_b1, C0 + C1, C1,
                           row_order=list(range(16)) + [-1] * 16 +
                           list(range(16, 48)))
    wd0, bd0 = _prep_weights(dec_w0, dec_b0, C1 + C1, C1,
                             row_order=list(range(64)))
    wd1, bd1 = _prep_weights(dec_w1, dec_b1, C1 + C0, C0,
                             row_order=[-1] * 32 + list(range(48)))
    fcv = np.concatenate(
        [np.asarray(fc_w, np.float32).reshape(C0),
         np.asarray(fc_b, np.float32).reshape(1)]).reshape(C0 + 1, 1)
    fcv = np.ascontiguousarray(fcv.astype(NP_BF16))

    in_maps = []
    for core in range(NCORES):
        xs = x[core * B:(core + 1) * B]      # [B, T, F]
        xp = np.zeros((T, B, SEG), np.float32)
        xp[:, :, 1:1 + F] = xs.transpose(1, 0, 2)
        in_maps.append({
            "x_pad": xp.astype(NP_BF16),
            "w0": w0, "w1": w1, "wd0": wd0, "wd1": wd1, "fcv": fcv,
            "b0": b0, "b1": b1, "bd0": bd0, "bd1": bd1,
        })

    _CACHE["in_maps"] = in_maps
    res = run_bass_kernel_spmd(nc, in_maps, core_ids=list(range(NCORES)))
    outs = [res.results[i]["out"] for i in range(NCORES)]
    return np.concatenate(outs, axis=0).astype(np.float32)


if __name__ == "__main__":
    rng = np.random.default_rng(0)
    inputs = {
        "x": rng.standard_normal((B_TOT, T, F), dtype=np.float32),
        "enc_w0": rng.standard_normal((4 * C0, 1 + C0, 3, 3), dtype=np.float32) * 0.05,
        "enc_b0": np.zeros(4 * C0, np.float32),
        "enc_w1": rng.standard_normal((4 * C1, C0 + C1, 3, 3), dtype=np.float32) * 0.05,
        "enc_b1": np.zeros(4 * C1, np.float32),
        "dec_w0": rng.standard_normal((4 * C1, C1 + C1, 3, 3), dtype=np.float32) * 0.05,
        "dec_b0": np.zeros(4 * C1, np.float32),
        "dec_w1": rng.standard_normal((4 * C0, C1 + C0, 3, 3), dtype=np.float32) * 0.05,
        "dec_b1": np.zeros(4 * C0, np.float32),
        "fc_w": rng.standard_normal((1, C0, 1, 1), dtype=np.float32) * 0.05,
        "fc_b": np.zeros(1, np.float32),
    }
    out = kernel(**inputs)
    print("out", out.shape, out.dtype, np.abs(out).max())


# revision 10
# speedup vs baseline: 4.5003x; 1.1104x over previous
import sys

sys.path.insert(0, "/opt/trn_rl_repo")

import numpy as np
import ml_dtypes

import concourse.bass as bass
import concourse.bass_isa as bass_isa
from concourse import bacc
import concourse.mybir as mybir
import concourse.tile as tile
from concourse.bass_utils import run_bass_kernel_spmd

# Problem constants (hardcoded; see nn_ConvLSTMAutoencoder spec)
B_TOT, T, F = 128, 100, 64
NCORES = 8
B = B_TOT // NCORES          # 16 batch per core (pure data parallelism)
SEG = F + 2                  # 66: spatial row stored with 1 zero pad each side
C0, C1 = 16, 32              # encoder hidden dims; decoder mirrors [32, 16]
NQ = (T + 2) // 3            # seq: 3 slots (bases 0/32/64) per column

F32 = mybir.dt.float32
BF16 = mybir.dt.bfloat16
NP_BF16 = ml_dtypes.bfloat16

Tanh = mybir.ActivationFunctionType.Tanh
MULT = mybir.AluOpType.mult
ADD = mybir.AluOpType.add

# Hardware rules (TRN2, probed via birverifier + perfetto): every
# compute-engine operand must start at partition 0/32/64/96; spans may not
# cross upward into a lower-numbered quadrant boundary (base 0: any count,
# base 64: up to 64, bases 32/96: up to 32). Both matmul operands must share
# the same partition base (any of the four). One-input ops (activation /
# tensor_scalar) may re-base, and are fast at any base. A 2-input DVE op
# whose SOURCES sit at base 32/96 runs ~12x slower; keep those at 0/64.
# gpsimd tensor ops cost ~15us each - never use them. DMA is unconstrained
# but a DMA writing a tile serializes against all compute reads of it, so
# per-step DMA targets must not be matmul-rhs tiles (that WAR lands on the
# recurrence spine).
#
# Layout:
#  arena_e rows: 0:16 h_enc0 | 16:32 zero | 32:64 h_enc1. x lives in its own
#    double-buffered tile; its conv contribution is PSUM-accumulated early by
#    separate K=1 matmuls, so no DMA ever writes arena_e.
#  arena_d rows: 0:32 h_dec0 | 32:48 h_dec1 | 48 ones (carries fc bias).
#    The decoder input e2(t) is read straight out of the padded seq buffer
#    (no per-step DMA); its weight rows are replicated at all 4 partition
#    bases so lhsT can match the cycling seq slot base.
#  seq: [128, NQ, B, SEG] padded (col 0 and 65 stay zero) so conv taps can
#    slide over it directly.
#
# Gate layout along M for each conv matmul, gates i/f/o pre-halved so one
# tanh serves all four (sigmoid(x) = 0.5*tanh(x/2)+0.5):
#   C=32: [i 0:32 | f 32:64 | o 64:96 | g 96:128]            M = 128
#   C=16: [i 0:16 |-| f 32:48 |-| o 64:80 |-| g 96:112]      M = 112
# Per cell (ACT = scalar engine, TS/TT = vector engine, all bf16):
#   th[0:M]      = Tanh(z + b)         ACT
#   s[0:64+C]    = 0.5*th+0.5          TS   (sig_i@0, sig_o@64)
#   sf[0:C]      = 0.5*th[32:+C]+0.5   TS   (re-base 32->0)
#   gh[0:C]      = th[96:+C]           TS   (re-base 96->0)
#   u = s[0:C]*gh; v = sf*c; c = u+v   TT   (base 0)
#   tc[64:+C]    = Tanh(c)             ACT  (re-base 0->64)
#   h = s[64:+C]*tc[64:+C]             TT   (base 64)

MM_NB = 8        # batches per matmul instruction (8*64 = 512 <= psum bank)


def _mm_taps(nc, zt, wt, rhs_src, start, stop):
    """3-tap conv contribution as PSUM-accumulated matmuls (bf16)."""
    for d in range(3):
        rhs = rhs_src(d)
        for nb in range(0, B, MM_NB):
            nc.tensor.matmul(
                zt[:, nb:nb + MM_NB, :],
                wt[:, d, :],
                rhs[:, nb:nb + MM_NB, :],
                start=(start and d == 0),
                stop=(stop and d == 2),
                skip_group_check=True,
            )


def _lstm_cell(nc, wpool, z, C, M, bvec, ctile, h_out):
    """Gate math for one ConvLSTM step under the alignment + perf rules."""
    th = wpool.tile([M, B, F], BF16, tag="th", name="th")
    nc.scalar.activation(th[:], z[0:M], Tanh, bias=bvec[0:M])
    s = wpool.tile([64 + C, B, F], BF16, tag="s", name="s")
    nc.vector.tensor_scalar(s[:], th[0:64 + C], 0.5, 0.5, MULT, ADD)
    sf = wpool.tile([C, B, F], BF16, tag="sf", name="sf")
    nc.vector.tensor_scalar(sf[:], th[32:32 + C], 0.5, 0.5, MULT, ADD)
    gh = wpool.tile([C, B, F], BF16, tag="gh", name="gh")
    nc.vector.tensor_scalar_mul(gh[:], th[96:96 + C], 1.0)
    u = wpool.tile([C, B, F], BF16, tag="u", name="u")
    nc.vector.tensor_tensor(u[:], s[0:C], gh[:], MULT)        # sig(i)*tanh(g)
    v = wpool.tile([C, B, F], BF16, tag="v", name="v")
    nc.vector.tensor_tensor(v[:], sf[:], ctile[:], MULT)      # sig(f)*c
    nc.vector.tensor_tensor(ctile[:], u[:], v[:], ADD)        # c updated
    tc_ = wpool.tile([64 + C, B, F], BF16, tag="tc", name="tc")
    nc.scalar.activation(tc_[64:64 + C], ctile[:], Tanh)
    nc.vector.tensor_tensor(h_out, s[64:64 + C], tc_[64:64 + C], MULT)


def build_program():
    nc = bacc.Bacc(None)

    x_pad = nc.declare_dram_parameter("x_pad", [T, B, SEG], BF16, isOutput=False)
    w0h = nc.declare_dram_parameter("w0h", [C0, 3, 112], BF16, isOutput=False)
    w0x = nc.declare_dram_parameter("w0x", [1, 3, 112], BF16, isOutput=False)
    w1 = nc.declare_dram_parameter("w1", [2 * C1, 3, 128], BF16, isOutput=False)
    wd0e = nc.declare_dram_parameter("wd0e", [96, 3, 128], BF16, isOutput=False)
    wd0h = nc.declare_dram_parameter("wd0h", [C1, 3, 128], BF16, isOutput=False)
    wd1 = nc.declare_dram_parameter("wd1", [C1 + C0, 3, 112], BF16, isOutput=False)
    fcv = nc.declare_dram_parameter("fcv", [C0 + 1, 1], BF16, isOutput=False)
    b0 = nc.declare_dram_parameter("b0", [112, 1], F32, isOutput=False)
    b1 = nc.declare_dram_parameter("b1", [128, 1], F32, isOutput=False)
    bd0 = nc.declare_dram_parameter("bd0", [128, 1], F32, isOutput=False)
    bd1 = nc.declare_dram_parameter("bd1", [112, 1], F32, isOutput=False)
    out = nc.declare_dram_parameter("out", [B, T, F], F32, isOutput=True)

    with tile.TileContext(nc) as tc:
        with (
            tc.tile_pool(name="const", bufs=1) as cpool,
            tc.tile_pool(name="state", bufs=1) as spool,
            tc.tile_pool(name="work", bufs=2) as wpool,
            tc.tile_pool(name="zp", bufs=1, space="PSUM") as zpool,
            tc.tile_pool(name="fcp", bufs=2, space="PSUM") as fcpool,
        ):
            w0ht = cpool.tile([C0, 3, 112], BF16)
            w0xt = cpool.tile([1, 3, 112], BF16)
            w1t = cpool.tile([2 * C1, 3, 128], BF16)
            wd0et = cpool.tile([96, 3, 128], BF16)
            wd0ht = cpool.tile([C1, 3, 128], BF16)
            wd1t = cpool.tile([C1 + C0, 3, 112], BF16)
            # fc weights staged at partitions 32:49 to match arena_d rows
            fcvt = cpool.tile([32 + C0 + 1, 1], BF16)
            b0t = cpool.tile([112, 1], F32)
            b1t = cpool.tile([128, 1], F32)
            bd0t = cpool.tile([128, 1], F32)
            bd1t = cpool.tile([112, 1], F32)
            for dst, dsrc in [(w0ht[:], w0h), (w0xt[:], w0x), (w1t[:], w1),
                              (wd0et[:], wd0e), (wd0ht[:], wd0h),
                              (wd1t[:], wd1), (fcvt[32:32 + C0 + 1], fcv),
                              (b0t[:], b0), (b1t[:], b1), (bd0t[:], bd0),
                              (bd1t[:], bd1)]:
                nc.sync.dma_start(dst, dsrc[:])

            arena_e = spool.tile([2 * C1, B, SEG], BF16)
            arena_d = spool.tile([C1 + C0 + 1, B, SEG], BF16)
            seq = spool.tile([96, NQ, B, SEG], BF16)
            nc.vector.memset(arena_e[:], 0.0)
            nc.vector.memset(arena_d[0:C1 + C0 + 1], 1.0)  # row 48 stays 1.0
            nc.vector.memset(arena_d[0:C1 + C0], 0.0)
            nc.vector.memset(seq[:], 0.0)

            # cell states at partition base 0 of their own tiles
            ce0 = spool.tile([C0, B, F], BF16)
            ce1 = spool.tile([C1, B, F], BF16)
            cd0 = spool.tile([C1, B, F], BF16)
            cd1 = spool.tile([C0, B, F], BF16)
            nc.vector.memset(ce0[:], 0.0)
            nc.vector.memset(ce1[:], 0.0)

            # ---------------- encoder ----------------
            # The x-part (K=1) matmuls for step t+1 are issued BEFORE z1(t)
            # so the in-order tensor engine runs them during cell0(t)'s gate
            # chain instead of blocking the spine behind h0(t).
            def _xpart(t):
                xr = wpool.tile([1, B, SEG], BF16, tag="xr", name="xr")
                nc.sync.dma_start(xr[:], x_pad[t:t + 1, :, :])
                z0n = zpool.tile([112, B, F], F32, tag="z112", name="z0")
                _mm_taps(nc, z0n, w0xt, lambda d: xr[0:1, :, d:d + F],
                         start=True, stop=False)
                return z0n

            z0 = _xpart(0)
            for t in range(T):
                _mm_taps(nc, z0, w0ht, lambda d: arena_e[0:C0, :, d:d + F],
                         start=False, stop=True)
                _lstm_cell(nc, wpool, z0, C0, 112, b0t, ce0,
                           arena_e[0:C0, :, 1:1 + F])

                if t + 1 < T:
                    z0 = _xpart(t + 1)

                z1 = zpool.tile([128, B, F], F32, tag="z128", name="z1")
                _mm_taps(nc, z1, w1t, lambda d: arena_e[0:2 * C1, :, d:d + F],
                         start=True, stop=True)
                _lstm_cell(nc, wpool, z1, C1, 128, b1t, ce1,
                           arena_e[C1:2 * C1, :, 1:1 + F])

                r = (t % 3) * 32
                nc.sync.dma_start(seq[r:r + 32, t // 3, :, 1:1 + F],
                                  arena_e[C1:2 * C1, :, 1:1 + F])

            # ---------------- decoder init ----------------
            nc.sync.dma_start(arena_d[0:C1, :, :], arena_e[C1:2 * C1, :, :])
            nc.sync.dma_start(arena_d[C1:C1 + C0, :, :], arena_e[0:C0, :, :])
            nc.sync.dma_start(cd0[:], ce1[:])
            nc.sync.dma_start(cd1[:], ce0[:])

            # ---------------- decoder ----------------
            # e2-part matmuls for step t+1 are issued early (their seq data
            # has been ready since the encoder phase); the fc for step t-1 is
            # issued before cell_d1(t) overwrites h_d1, so fc never blocks
            # the next zd0 block on the in-order tensor engine.
            def _e2part(t):
                r = (t % 3) * 32
                zd0n = zpool.tile([128, B, F], F32, tag="z128", name="zd0")
                _mm_taps(nc, zd0n, wd0et[r:r + 32],
                         lambda d: seq[r:r + 32, t // 3, :, d:d + F],
                         start=True, stop=False)
                return zd0n

            def _fc(t):
                # final 1x1 conv: K=17, M=1 matmul read at partition base 32
                # (ones row carries fc_b), then DVE copy out of PSUM.
                zfc = fcpool.tile([1, B, F], F32, tag="fc", name="zfc")
                for nb in range(0, B, MM_NB):
                    nc.tensor.matmul(
                        zfc[:, nb:nb + MM_NB, :],
                        fcvt[32:32 + C0 + 1],
                        arena_d[C1:C1 + C0 + 1, nb:nb + MM_NB, 1:1 + F],
                        start=True, stop=True)
                ofc = wpool.tile([1, B, F], F32, tag="ofc", name="ofc")
                nc.vector.tensor_scalar_mul(ofc[:], zfc[:], 1.0)
                nc.sync.dma_start(out[:, t, :], ofc[0:1, :, :])

            zd0 = _e2part(0)
            for t in range(T):
                _mm_taps(nc, zd0, wd0ht,
                         lambda d: arena_d[0:C1, :, d:d + F],
                         start=False, stop=True)
                _lstm_cell(nc, wpool, zd0, C1, 128, bd0t, cd0,
                           arena_d[0:C1, :, 1:1 + F])

                if t + 1 < T:
                    zd0 = _e2part(t + 1)

                zd1 = zpool.tile([112, B, F], F32, tag="z112", name="zd1")
                _mm_taps(nc, zd1, wd1t,
                         lambda d: arena_d[0:C1 + C0, :, d:d + F],
                         start=True, stop=True)
                if t > 0:
                    _fc(t - 1)
                _lstm_cell(nc, wpool, zd1, C0, 112, bd1t, cd1,
                           arena_d[C1:C1 + C0, :, 1:1 + F])
            _fc(T - 1)

    nc.finalize()
    return nc


# M-column spread per gate, by hidden size
def _m_cols(C):
    return {"i": 0, "f": 32, "o": 64, "g": 96}, 96 + C


def _prep_weights(w, b, Cin, C, row_order):
    """[4C, Cin, 3, 3] -> lhsT [len(row_order), 3, M] with the gate spread.

    Reference gate order along output channels is i, f, o, g. The i/f/o rows
    (and biases) are halved so one tanh serves all gates. row_order maps
    lhsT row -> input channel (-1 = zero row).
    """
    cols, M = _m_cols(C)
    w3 = np.asarray(w, np.float32).reshape(4 * C, Cin, 3, 3)[:, :, :, 1]
    b = np.asarray(b, np.float32).reshape(4 * C)
    gate_of = {"i": 0, "f": 1, "o": 2, "g": 3}
    lhsT = np.zeros((len(row_order), 3, M), np.float32)
    bvec = np.zeros((M, 1), np.float32)
    for gname, col0 in cols.items():
        gi = gate_of[gname]
        scale = 0.5 if gname in ("i", "f", "o") else 1.0
        for j in range(C):
            oc = gi * C + j
            bvec[col0 + j, 0] = b[oc] * scale
            for r, ch in enumerate(row_order):
                if ch >= 0:
                    lhsT[r, :, col0 + j] = w3[oc, ch, :] * scale
    return np.ascontiguousarray(lhsT).astype(NP_BF16), bvec


_CACHE = {}


def kernel(x, enc_w0, enc_b0, enc_w1, enc_b1, dec_w0, dec_b0, dec_w1, dec_b1,
           fc_w, fc_b):
    if "nc" not in _CACHE:
        _CACHE["nc"] = build_program()
    nc = _CACHE["nc"]

    x = np.asarray(x, np.float32)
    # enc0: channel 0 = x (own tile), channels 1..16 = h0 (arena rows 0:16)
    w0full, b0 = _prep_weights(enc_w0, enc_b0, 1 + C0, C0,
                               row_order=[0] + list(range(1, 17)))
    w0x = np.ascontiguousarray(w0full[0:1])
    w0h = np.ascontiguousarray(w0full[1:17])
    # enc1 rhs rows 0:64: h0 (ch 0..15), 16 zero rows, h1 (ch 16..47)
    w1, b1 = _prep_weights(enc_w1, enc_b1, C0 + C1, C1,
                           row_order=list(range(16)) + [-1] * 16 +
                           list(range(16, 48)))
    # dec0: e2 part (ch 0..31) replicated at all 4 partition bases;
    # h_d0 part (ch 32..63) at base 0
    wd0full, bd0 = _prep_weights(dec_w0, dec_b0, C1 + C1, C1,
                                 row_order=list(range(64)))
    wd0e = np.ascontiguousarray(np.tile(wd0full[0:32], (3, 1, 1)))
    wd0h = np.ascontiguousarray(wd0full[32:64])
    # dec1 rhs rows 0:48: h_d0 (ch 0..31), h_d1 (ch 32..47)
    wd1, bd1 = _prep_weights(dec_w1, dec_b1, C1 + C0, C0,
                             row_order=list(range(48)))
    fcv = np.concatenate(
        [np.asarray(fc_w, np.float32).reshape(C0),
         np.asarray(fc_b, np.float32).reshape(1)]).reshape(C0 + 1, 1)
    fcv = np.ascontiguousarray(fcv.astype(NP_BF16))

    in_maps = []
    for core in range(NCORES):
        xs = x[core * B:(core + 1) * B]      # [B, T, F]
        xp = np.zeros((T, B, SEG), np.float32)
        xp[:, :, 1:1 + F] = xs.transpose(1, 0, 2)
        in_maps.append({
            "x_pad": xp.astype(NP_BF16),
            "w0h": w0h, "w0x": w0x, "w1": w1,
            "wd0e": wd0e, "wd0h": wd0h, "wd1": wd1, "fcv": fcv,
            "b0": b0, "b1": b1, "bd0": bd0, "bd1": bd1,
        })

    _CACHE["in_maps"] = in_maps
    res = run_bass_kernel_spmd(nc, in_maps, core_ids=list(range(NCORES)))
    outs = [res.results[i]["out"] for i in range(NCORES)]
    return np.concatenate(outs, axis=0).astype(np.float32)


if __name__ == "__main__":
    rng = np.random.default_rng(0)
    inputs = {
        "x": rng.standard_normal((B_TOT, T, F), dtype=np.float32),
        "enc_w0": rng.standard_normal((4 * C0, 1 + C0, 3, 3), dtype=np.float32) * 0.05,
        "enc_b0": np.zeros(4 * C0, np.float32),
        "enc_w1": rng.standard_normal((4 * C1, C0 + C1, 3, 3), dtype=np.float32) * 0.05,
        "enc_b1": np.zeros(4 * C1, np.float32),
        "dec_w0": rng.standard_normal((4 * C1, C1 + C1, 3, 3), dtype=np.float32) * 0.05,
        "dec_b0": np.zeros(4 * C1, np.float32),
        "dec_w1": rng.standard_normal((4 * C0, C1 + C0, 3, 3), dtype=np.float32) * 0.05,
        "dec_b1": np.zeros(4 * C0, np.float32),
        "fc_w": rng.standard_normal((1, C0, 1, 1), dtype=np.float32) * 0.05,
        "fc_b": np.zeros(1, np.float32),
    }
    out = kernel(**inputs)
    print("out", out.shape, out.dtype, np.abs(out).max())
